# revision 1
# baseline (speedup 1.0000x reference)
"""Trainium2 Bass kernel for ClusterSeparationOptimizer.

Math (verified exactly vs reference):
  signed[i,n,j,h] = [x, y, 1] @ (A_i @ W[:, j, h])   (affine in the RAW point)
  mn = min_h signed,  nmx = -max_h signed            (over valid edges)
  v  = max(mn, nmx)
  viol = (v >= -EPS) * max(sigmoid(v), 0.5) * cluster_mask
  out  = sum viol (i!=j, hull_ok) + 0.1*|translations|^2 + |angles|^2

Geometric pruning (exact, not approximate): each cluster's points are
kd-sorted into 12 chunks of 128 with tight bboxes. signed is affine in the
raw point, so its extrema over a chunk's bbox are attained at the 4 box
corners; if some edge has all corners < -tau and another has all corners
> +tau (tau >> EPS), every point of the chunk has v < -EPS and the whole
(chunk, hull) pair contributes exactly 0. Only ~8% of pairs survive.
Degenerate hulls (no valid edges -> reference viol=1 everywhere) are never
pruned.

Device kernel (SPMD, one program, per-core data): T tiles, each tile =
(128 points, SLOTS hull-slots) computed by one K=3 fp32 matmul
([x,y,1] stationary, G-columns moving). Matmuls are packed 4-way into
distinct PE row-groups via tile_position (points and G live in 4
partition bands at 32*q), with outputs striped across the 4 banks of a
16-tile PSUM group so concurrent quads never share a bank. The (bank,
block) PSUM positions form a uniform stride-128 sequence, so one
tensor_reduce(min) + one tensor_reduce(max, negate=True) with a 4D access
pattern cover a whole group. A batched tail (TT max, ACT sigmoid, is_ge
gate, tensor_scalar, two multiplies) lands in an SBUF strip; a final
reduce_sum + ones-matmul gives the scalar partial. Hull-slot padding uses
+-BIG poison columns (v=-BIG -> gated to 0). Per-core scalars are summed
on the host (the all-reduce), which also adds the tiny penalty terms.
"""

import numpy as np

C, N, H = 24, 1536, 40
NCORES = 8
PCHUNK = 128
NCHUNK = N // PCHUNK       # 12 chunks per cluster
SLOTS = 2                  # hull j-slots per tile
TW = SLOTS * H             # 120 free columns per tile
PSTRIDE = 128              # PSUM cols reserved per tile (within one bank)
GRP = 16                   # tiles per 4-bank PSUM group: bank=k%4, block=k//4
QROW = 4                   # PE row-group packing ways (tile_position)
SEP_W, T_PEN, R_PEN = 1.0, 0.1, 1.0
EPS = 1e-8
BIG = 1e30
MARGIN = 1e-2

_NC_CACHE = {}


def _transform64(x, med, ang, tr):
    c, s = np.cos(ang), np.sin(ang)
    xc = x[..., 0] - med[:, None, 0]
    yc = x[..., 1] - med[:, None, 1]
    px = c[:, None] * xc - s[:, None] * yc + (med[:, 0] + tr[:, 0])[:, None]
    py = s[:, None] * xc + c[:, None] * yc + (med[:, 1] + tr[:, 1])[:, None]
    return np.stack([px, py], -1)


def _host_coeffs(ph, med, ang, tr, hm):
    """G[i] = A_i @ W: (C, 3, C, H) float64; rows act on [x, y, 1]."""
    hulT = _transform64(ph, med, ang, tr)
    hx, hy = hulT[..., 0], hulT[..., 1]
    ex = np.roll(hx, -1, axis=1) - hx
    ey = np.roll(hy, -1, axis=1) - hy
    elen_raw = np.sqrt(ex * ex + ey * ey)
    elen = elen_raw + EPS
    evalid = elen_raw > 1e-6
    a = ex / elen
    b = -ey / elen
    d = -(ex * hy - ey * hx) / elen

    W = np.stack([b, a, d], axis=0)  # (3, C, H): coefficients on [px', py', 1]
    degenerate = np.zeros(C, bool)
    for j in range(C):
        inv = ~evalid[j]
        if inv.any():
            val = np.nonzero(evalid[j])[0]
            if len(val) > 0:
                W[:, j, inv] = W[:, j, val[-1]][:, None]
            else:
                # no valid edges: reference yields viol=1 (min over empty=inf)
                W[:, j, :] = np.array([0.0, 0.0, BIG])[:, None]
                degenerate[j] = True

    c, s = np.cos(ang), np.sin(ang)
    A = np.zeros((C, 3, 3))
    A[:, 0, 0] = c
    A[:, 0, 1] = s
    A[:, 1, 0] = -s
    A[:, 1, 1] = c
    A[:, 2, 0] = med[:, 0] + tr[:, 0] - c * med[:, 0] + s * med[:, 1]
    A[:, 2, 1] = med[:, 1] + tr[:, 1] - s * med[:, 0] - c * med[:, 1]
    A[:, 2, 2] = 1.0

    G = np.einsum("ikl,lm->ikm", A, W.reshape(3, C * H))
    return G.reshape(C, 3, C, H), hulT, degenerate


def _kd_chunks(p):
    """Split points into 12 chunks of 128 via 3x2x2 median splits."""
    def split(ids, parts):
        if parts == 1:
            return [ids]
        q = p[ids]
        dim = 0 if np.ptp(q[:, 0]) >= np.ptp(q[:, 1]) else 1
        order = ids[np.argsort(q[:, dim], kind="stable")]
        if parts % 3 == 0:
            k = len(order) // 3
            return (split(order[:k], parts // 3)
                    + split(order[k:2 * k], parts // 3)
                    + split(order[2 * k:], parts // 3))
        k = len(order) // 2
        return split(order[:k], parts // 2) + split(order[k:], parts // 2)
    return split(np.arange(len(p)), NCHUNK)


_POISON = np.zeros((3, H))
_POISON[2, : H // 2] = BIG
_POISON[2, H // 2:] = -BIG


def _plan_and_pack(pc, ph, med, ang, tr, cm, hm):
    """Returns (T_prog, in_maps): per-core packed inputs."""
    med64 = med.astype(np.float64)
    ang64 = ang.astype(np.float64)
    tr64 = tr.astype(np.float64)
    G, hulT, degen = _host_coeffs(ph.astype(np.float64), med64, ang64, tr64, hm)
    ptsT = _transform64(pc.astype(np.float64), med64, ang64, tr64)  # (C,N,2)
    hull_ok = hm.sum(-1) >= 3

    # Exact prune test on RAW-point bboxes: signed[n,h] = [x,y,1] @ G[i,:,jh]
    # is affine in the raw point, so its min/max over a box are attained at
    # the 4 corners. If some edge has all corners < -tau and some edge has
    # all corners > +tau (tau >> EPS), then for EVERY point of the box
    # mn < -EPS and mx > EPS, hence v < -EPS and viol == 0. No convexity or
    # orientation assumptions needed.
    TAU = 1e-5

    # tiles: (cluster i, point-index array (128,), list of <=SLOTS j's)
    tiles = []
    for i in range(C):
        for ch in _kd_chunks(ptsT[i]):
            q = pc[i, ch].astype(np.float64)
            qmin, qmax = q.min(0), q.max(0)
            corners = np.array([[qmin[0], qmin[1], 1.0], [qmin[0], qmax[1], 1.0],
                                [qmax[0], qmin[1], 1.0], [qmax[0], qmax[1], 1.0]])
            # s_corners: (4, C, H)
            sc = np.einsum("ck,ikjh->cijh", corners,
                           G[i].reshape(1, 3, C, H))[:, 0]
            neg_edge = (sc.max(0) < -TAU).any(-1)   # (C,) some edge all-neg
            pos_edge = (sc.min(0) > TAU).any(-1)    # (C,) some edge all-pos
            prunable = neg_edge & pos_edge
            # degenerate hulls (no valid edges) contribute viol=1 for every
            # point regardless of position -- never prune them
            alive = [j for j in range(C)
                     if j != i and hull_ok[j]
                     and (degen[j] or not prunable[j])]
            for k in range(0, len(alive), SLOTS):
                tiles.append((i, ch, alive[k:k + SLOTS]))

    # distribute: largest-first round-robin is unnecessary (all tiles equal
    # cost) -- just deal them out evenly
    percore = [tiles[c::NCORES] for c in range(NCORES)]
    T = max(1, max(len(p) for p in percore))
    T_prog = ((T + GRP // 2 - 1) // (GRP // 2)) * (GRP // 2)

    TQ = T_prog // QROW
    in_maps = []
    for c in range(NCORES):
        # tile t lives in partition band 32*(t%QROW), column block t//QROW
        pts3 = np.zeros((PCHUNK, TQ * PCHUNK), np.float32)
        for qb in range(QROW):
            pts3[32 * qb + 2] = 1.0
        gco = np.zeros((PCHUNK, TQ * TW), np.float64)
        for qb in range(QROW):
            gco[32 * qb: 32 * qb + 3] = np.tile(_POISON, (1, TQ * SLOTS))
        cm3 = np.zeros((PCHUNK, T_prog * SLOTS), np.float32)
        for t, (i, ch, js) in enumerate(percore[c]):
            qb, blk = t % QROW, t // QROW
            pts3[32 * qb + 0, blk * PCHUNK:(blk + 1) * PCHUNK] = pc[i, ch, 0]
            pts3[32 * qb + 1, blk * PCHUNK:(blk + 1) * PCHUNK] = pc[i, ch, 1]
            for s_, j in enumerate(js):
                gco[32 * qb: 32 * qb + 3,
                    blk * TW + s_ * H: blk * TW + (s_ + 1) * H] = G[i, :, j, :]
                cm3[:, _strip_col(t, T_prog) + s_] = cm[i, ch]
        in_maps.append({
            "pts3": np.ascontiguousarray(pts3),
            "gcoef": np.ascontiguousarray(gco.astype(np.float32)),
            "cmask": np.ascontiguousarray(cm3),
        })
    return T_prog, in_maps


def _strip_col(t, T_prog):
    """mn/nmx/v strip column of tile t's first slot. Within a group of
    gsz tiles, reduces run per PSUM bank b = k%4 over blocks k//4, so the
    strip is ordered (bank, block) within the group."""
    g, k = t // GRP, t % GRP
    gsz = min(GRP, T_prog - g * GRP)
    return (g * GRP + (k % 4) * (gsz // 4) + k // 4) * SLOTS


def _build_nc(T_prog, reps=1, loop=None):
    import concourse.bacc as bacc
    import concourse.mybir as mybir
    from concourse.tile import TileContext

    f32 = mybir.dt.float32
    nc = bacc.Bacc()

    TQ = (T_prog + QROW - 1) // QROW
    pts_d = nc.dram_tensor("pts3", [PCHUNK, TQ * PCHUNK], f32, kind="ExternalInput")
    g_d = nc.dram_tensor("gcoef", [PCHUNK, TQ * TW], f32, kind="ExternalInput")
    cm_d = nc.dram_tensor("cmask", [PCHUNK, T_prog * SLOTS], f32, kind="ExternalInput")
    out_d = nc.dram_tensor("out", [1, 1], f32, kind="ExternalOutput")

    group_sizes = [GRP] * (T_prog // GRP)
    if T_prog % GRP:
        group_sizes.append(T_prog % GRP)
    GW = GRP * SLOTS  # strip columns per full group

    with TileContext(nc) as tc:
        with tc.tile_pool(name="const", bufs=1) as cpool, \
             tc.tile_pool(name="work", bufs=4) as wpool, \
             tc.tile_pool(name="psum", bufs=2, space="PSUM") as ppool:

            sp = mybir.EngineType.SP
            pts_sb = cpool.tile_from(pts_d[:, :], forced_dma_engine=sp)
            g_sb = cpool.tile_from(g_d[:, :], forced_dma_engine=sp)
            cm_sb = cpool.tile_from(cm_d[:, :], forced_dma_engine=sp)
            mnstrip = cpool.tile([PCHUNK, T_prog * SLOTS], f32)
            nmxstrip = cpool.tile([PCHUNK, T_prog * SLOTS], f32)
            vstrip = cpool.tile([PCHUNK, T_prog * SLOTS], f32)
            ones_sb = cpool.tile([PCHUNK, 1], f32)
            nc.vector.memset(ones_sb, 1.0)

            def body():
                t0 = 0
                for gsz in group_sizes:
                    assert gsz % 4 == 0
                    ps = ppool.tile([PCHUNK, GRP * PSTRIDE], f32, tag="ps")
                    for k in range(gsz):
                        t = t0 + k
                        qb, blk = t % QROW, t // QROW
                        # bank = k%4, block-within-bank = k//4
                        off = (k % 4) * 512 + (k // 4) * PSTRIDE
                        nc.tensor.matmul(
                            ps[:, off: off + TW],
                            pts_sb[32 * qb: 32 * qb + 3,
                                   blk * PCHUNK:(blk + 1) * PCHUNK],
                            g_sb[32 * qb: 32 * qb + 3,
                                 blk * TW:(blk + 1) * TW],
                            start=True, stop=True,
                            tile_position=(32 * qb, 0),
                        )
                    if gsz == GRP:
                        # positions (k%4)*512+(k//4)*128 cover stride-128
                        # uniformly; order matches _strip_col
                        view = ps.rearrange("p (e r) -> p e r", e=GRP)[:, :, 0:TW] \
                                 .rearrange("p e (s h) -> p e s h", h=H)
                        so = t0 * SLOTS
                        sw = GRP * SLOTS
                        nc.vector.tensor_reduce(
                            out=mnstrip[:, so:so + sw], in_=view,
                            axis=mybir.AxisListType.X, op=mybir.AluOpType.min,
                        )
                        nc.vector.tensor_reduce(
                            out=nmxstrip[:, so:so + sw], in_=view,
                            axis=mybir.AxisListType.X, op=mybir.AluOpType.max,
                            negate=True,
                        )
                    else:
                        nbk = gsz // 4
                        for b in range(4):
                            view = ps[:, b * 512: b * 512 + nbk * PSTRIDE] \
                                     .rearrange("p (e r) -> p e r", e=nbk)[:, :, 0:TW] \
                                     .rearrange("p e (s h) -> p e s h", h=H)
                            so = (t0 + b * nbk) * SLOTS
                            sw = nbk * SLOTS
                            nc.vector.tensor_reduce(
                                out=mnstrip[:, so:so + sw], in_=view,
                                axis=mybir.AxisListType.X, op=mybir.AluOpType.min,
                            )
                            nc.vector.tensor_reduce(
                                out=nmxstrip[:, so:so + sw], in_=view,
                                axis=mybir.AxisListType.X, op=mybir.AluOpType.max,
                                negate=True,
                            )
                    t0 += gsz
                # batched tail over the full strips
                v = wpool.tile([PCHUNK, T_prog * SLOTS], f32)
                nc.vector.tensor_tensor(
                    out=v, in0=mnstrip, in1=nmxstrip, op=mybir.AluOpType.max)
                w = wpool.tile([PCHUNK, T_prog * SLOTS], f32)
                nc.scalar.activation(
                    out=w, in_=v, func=mybir.ActivationFunctionType.Sigmoid)
                g01 = wpool.tile([PCHUNK, T_prog * SLOTS], f32)
                nc.vector.tensor_scalar(
                    out=g01, in0=v, scalar1=-float(EPS), scalar2=None,
                    op0=mybir.AluOpType.is_ge)
                q = wpool.tile([PCHUNK, T_prog * SLOTS], f32)
                nc.vector.tensor_scalar(
                    out=q, in0=w, scalar1=0.5, scalar2=None,
                    op0=mybir.AluOpType.max)
                qq = wpool.tile([PCHUNK, T_prog * SLOTS], f32)
                nc.vector.tensor_tensor(
                    out=qq, in0=q, in1=g01, op=mybir.AluOpType.mult)
                nc.vector.tensor_tensor(
                    out=vstrip, in0=qq, in1=cm_sb,
                    op=mybir.AluOpType.mult)

            if loop is not None:
                import os as _os
                stg = _os.environ.get("LOOP_STAGGERED", "0") == "1"
                with tc.For_i(0, loop, 1, staggered_reset=stg) as _i:
                    body()
            else:
                for rep in range(reps):
                    body()

            acc = cpool.tile([PCHUNK, 1], f32)
            nc.vector.tensor_reduce(
                out=acc, in_=vstrip, axis=mybir.AxisListType.X,
                op=mybir.AluOpType.add,
            )
            out_ps = ppool.tile([1, 1], f32, tag="ps")
            nc.tensor.matmul(out_ps, acc, ones_sb, start=True, stop=True)
            out_sb = cpool.tile([1, 1], f32)
            nc.scalar.copy(out=out_sb, in_=out_ps)
            nc.sync.dma_start(out=out_d[:, :], in_=out_sb)

    nc.compile()  # Bacc passes: wait legalization, reg alloc, nop fusion
    return nc


def kernel(padded_clusters, padded_hulls, medoids, rotation_angles,
           translations, cluster_masks, hull_masks):
    pc = np.asarray(padded_clusters, dtype=np.float32)
    ph = np.asarray(padded_hulls, dtype=np.float32)
    med = np.asarray(medoids, dtype=np.float32)
    ang = np.asarray(rotation_angles, dtype=np.float32)
    tr = np.asarray(translations, dtype=np.float32)
    cm = np.asarray(cluster_masks)
    hm = np.asarray(hull_masks)

    T_prog, in_maps = _plan_and_pack(pc, ph, med, ang, tr, cm, hm)

    key = ("nc", T_prog)
    if key not in _NC_CACHE:
        _NC_CACHE[key] = _build_nc(T_prog)
    nc = _NC_CACHE[key]

    from concourse.bass_utils import run_bass_kernel_spmd
    res = run_bass_kernel_spmd(nc, in_maps, core_ids=list(range(NCORES)))
    _NC_CACHE["last_results"] = res

    sep = sum(float(r["out"][0, 0]) for r in res.results)
    total = (SEP_W * sep
             + T_PEN * float(np.sum(tr.astype(np.float64) ** 2))
             + R_PEN * float(np.sum(ang.astype(np.float64) ** 2)))
    return np.asarray(total, dtype=np.float32)



# revision 2
# speedup vs baseline: 1.3383x; 1.3383x over previous
"""Trainium2 Bass kernel for ClusterSeparationOptimizer (v2).

Math (identical to reference up to fp32 rounding):
  signed[i,n,j,h] = [x, y, 1] @ (A_i @ W[:, j, h])   (affine in the RAW point)
  mn = min_h signed, mx = max_h signed               (over valid edges)
  v  = max(mn, -mx)   -> v >= -EPS iff inside; v = min|signed| when inside
  viol = sigmoid(v) * (v >= -EPS) * cluster_mask
  out  = sum viol (i!=j, hull_ok) + 0.1*|translations|^2 + |angles|^2

Host-side planning (fp64, exact):
  * Only VALID points are packed: each cluster's n_i real points are
    kd-split into ceil(n_i/128) chunks of <=128; chunks padded to 128 with
    far sentinels (cmask=0, v<0 there by convexity).
  * Hull orientation is normalized (W flipped so interior => all s > 0).
  * Per (chunk, hull) pair, exact corner tests on the chunk bbox (signed is
    affine in the raw point; env_lo=min_h s is concave so its box-min is at
    a corner):
      - pruned   : some edge all-corners < -TAU and some all > TAU
                   -> every point sign-mixed -> viol == 0.
      - deep     : env_lo >= DEEP at all corners -> sigmoid(mn) = 1 within
                   e^-DEEP per point; host adds count*1.0, pair skipped.
      - interior : env_lo >= POSM at all corners -> mx > 0 > -mx <= mn, so
                   v = mn exactly; the device skips the max-reduce.
      - boundary : both reduces.

Device (SPMD one program, per-core data):
  Pairs are packed as 40-wide column slots, 12 per PSUM bank.  One
  float32r matmul per bank: lhsT[K<=36,128] holds [x,y,1] of each slot's
  chunk K-triple (block-diagonal rhs holds each slot's 40 G columns), so a
  single wide (480-col, >=256 => 1 cycle/row) matmul computes 12 slots'
  signed distances for 128 points.  Banks are processed in 2 groups of 3
  with a bufs=2 PSUM pool so group g+1 matmuls overlap group g reduces.
  Per bank: DVE tensor_reduce(min) -> mn strip; for the first mxbank banks
  (boundary slots first) tensor_reduce(max, negate) -> -mx strip; the
  interior tail of the -mx strip is pre-set to -BIG once.  Tail:
  v = max(mn, nmx); sigmoid on ACT; (v >= -EPS) gate; * cmask -> vstrip.
  Final: reduce_sum + ones-matmul -> scalar; host all-reduces the 8 cores
  and adds the deep-interior count and penalty terms.
"""

import math

import numpy as np

C, N, H = 24, 1536, 40
NCORES = 8
P = 128                    # points per chunk / partition dim
SPB = 12                   # slots per 512-col PSUM bank (12*40=480)
BANKW = 512
SEP_W, T_PEN, R_PEN = 1.0, 0.1, 1.0
EPS = 1e-8
BIG = 1e30
TAU = 1e-5                 # prune margin
POSM = 1e-2                # interior margin (device fp32 slop ~1e-4)
DEEP = 8.5                 # deep-interior skip: per-point err <= e^-8.5
SENT = 1.0e6               # sentinel coordinate for padded points

_NC_CACHE = {}


def _transform64(x, med, ang, tr):
    c, s = np.cos(ang), np.sin(ang)
    xc = x[..., 0] - med[:, None, 0]
    yc = x[..., 1] - med[:, None, 1]
    px = c[:, None] * xc - s[:, None] * yc + (med[:, 0] + tr[:, 0])[:, None]
    py = s[:, None] * xc + c[:, None] * yc + (med[:, 1] + tr[:, 1])[:, None]
    return np.stack([px, py], -1)


def _host_coeffs(ph, med, ang, tr, hm):
    """G[i] = A_i @ W: (C, 3, C, H) float64; rows act on raw [x, y, 1].

    W is orientation-normalized so that hull interiors have s > 0."""
    hulT = _transform64(ph, med, ang, tr)
    hx, hy = hulT[..., 0], hulT[..., 1]
    ex = np.roll(hx, -1, axis=1) - hx
    ey = np.roll(hy, -1, axis=1) - hy
    elen_raw = np.sqrt(ex * ex + ey * ey)
    elen = elen_raw + EPS
    evalid = elen_raw > 1e-6
    a = ex / elen
    b = -ey / elen
    d = -(ex * hy - ey * hx) / elen

    W = np.stack([b, a, d], axis=0)  # (3, C, H): coeffs on transformed [x,y,1]
    degenerate = np.zeros(C, bool)
    flip = np.ones(C)
    for j in range(C):
        inv = ~evalid[j]
        val = np.nonzero(evalid[j])[0]
        if inv.any():
            if len(val) > 0:
                W[:, j, inv] = W[:, j, val[-1]][:, None]
            else:
                W[:, j, :] = np.array([0.0, 0.0, BIG])[:, None]
                degenerate[j] = True
        if not degenerate[j]:
            vm = hm[j] if hm[j].any() else np.ones(H, bool)
            cx, cy = hulT[j, vm, 0].mean(), hulT[j, vm, 1].mean()
            sc = W[0, j, val] * cx + W[1, j, val] * cy + W[2, j, val]
            if np.median(sc) < 0:
                flip[j] = -1.0
                W[:, j, :] = -W[:, j, :]

    c, s = np.cos(ang), np.sin(ang)
    A = np.zeros((C, 3, 3))
    A[:, 0, 0] = c
    A[:, 0, 1] = s
    A[:, 1, 0] = -s
    A[:, 1, 1] = c
    A[:, 2, 0] = med[:, 0] + tr[:, 0] - c * med[:, 0] + s * med[:, 1]
    A[:, 2, 1] = med[:, 1] + tr[:, 1] - s * med[:, 0] - c * med[:, 1]
    A[:, 2, 2] = 1.0

    G = np.einsum("ikl,lm->ikm", A, W.reshape(3, C * H))
    return G.reshape(C, 3, C, H), hulT, degenerate


def _kd_split(p, ids, parts):
    """Split index array ids into `parts` groups of near-equal size (each
    <= ceil(len/parts)) by recursive median cuts on the wider dimension."""
    if parts == 1:
        return [ids]
    q = p[ids]
    dim = 0 if np.ptp(q[:, 0]) >= np.ptp(q[:, 1]) else 1
    order = ids[np.argsort(q[:, dim], kind="stable")]
    pl = parts // 2
    k = (len(order) * pl + parts - 1) // parts
    return _kd_split(p, order[:k], pl) + _kd_split(p, order[k:], parts - pl)


def _plan_and_pack(pc, ph, med, ang, tr, cm, hm):
    """Returns (cfg, in_maps): cfg=(nbank, mxbank, host_extra)."""
    med64 = med.astype(np.float64)
    ang64 = ang.astype(np.float64)
    tr64 = tr.astype(np.float64)
    G, hulT, degen = _host_coeffs(ph.astype(np.float64), med64, ang64, tr64, hm)
    hull_ok = hm.sum(-1) >= 3

    host_deep = 0.0
    boundary = []   # (i, chunk_pts_idx (np array of <=128), j)
    interior = []
    for i in range(C):
        valid = np.nonzero(cm[i])[0]
        if len(valid) == 0:
            continue
        parts = (len(valid) + P - 1) // P
        Gi = G[i].reshape(3, C * H)
        for ch in _kd_split(pc[i].astype(np.float64), valid, parts):
            q = pc[i, ch].astype(np.float64)
            qmin, qmax = q.min(0), q.max(0)
            corners = np.array(
                [[qmin[0], qmin[1], 1.0], [qmin[0], qmax[1], 1.0],
                 [qmax[0], qmin[1], 1.0], [qmax[0], qmax[1], 1.0]])
            sc = (corners @ Gi).reshape(4, C, H)
            neg_edge = (sc.max(0) < -TAU).any(-1)
            pos_edge = (sc.min(0) > TAU).any(-1)
            prunable = neg_edge & pos_edge
            env_lo_min = sc.min(-1).min(0)          # (C,) box-min of min_h s
            for j in range(C):
                if j == i or not hull_ok[j]:
                    continue
                if not degen[j] and prunable[j]:
                    continue
                if degen[j] or env_lo_min[j] >= DEEP:
                    host_deep += float(len(ch))
                    continue
                if env_lo_min[j] >= POSM:
                    interior.append((i, ch, j))
                else:
                    boundary.append((i, ch, j))

    per_b = [boundary[c::NCORES] for c in range(NCORES)]
    per_i = [interior[c::NCORES] for c in range(NCORES)]
    max_b = max(len(x) for x in per_b)
    max_s = max(len(b) + len(t) for b, t in zip(per_b, per_i))
    mxbank = (max_b + SPB - 1) // SPB
    nbank = max((max_s + SPB - 1) // SPB, mxbank)
    nbank += nbank % 2        # even # banks -> two equal groups
    assert nbank <= 6, f"PSUM budget exceeded: nbank={nbank}"

    in_maps = []
    for c in range(NCORES):
        slots = per_b[c] + per_i[c]
        lhs = np.zeros((P, nbank * P), np.float32)
        rhs = np.zeros((P, nbank * BANKW), np.float32)
        cm3 = np.zeros((P, nbank * SPB), np.float32)
        for b in range(nbank):
            tri = {}
            for si, (i, ch, j) in enumerate(slots[b * SPB:(b + 1) * SPB]):
                key = (i, ch.tobytes())
                if key not in tri:
                    t = tri[key] = len(tri)
                    n = len(ch)
                    lhs[3 * t + 0, b * P: b * P + n] = pc[i, ch, 0]
                    lhs[3 * t + 1, b * P: b * P + n] = pc[i, ch, 1]
                    lhs[3 * t + 0, b * P + n:(b + 1) * P] = SENT
                    lhs[3 * t + 1, b * P + n:(b + 1) * P] = SENT
                    lhs[3 * t + 2, b * P:(b + 1) * P] = 1.0
                t = tri[key]
                co = b * BANKW + si * H
                rhs[3 * t: 3 * t + 3, co: co + H] = G[i, :, j, :]
                cm3[: len(ch), b * SPB + si] = 1.0
        in_maps.append({
            "lhs": np.ascontiguousarray(lhs),
            "rhs": np.ascontiguousarray(rhs),
            "cmask": np.ascontiguousarray(cm3),
        })
    return (nbank, mxbank, host_deep), in_maps


def _build_nc(cfg, reps=1, loop=None):
    import concourse.bacc as bacc
    import concourse.mybir as mybir
    from concourse.tile import TileContext

    nbank, mxbank = cfg[0], cfg[1]
    f32 = mybir.dt.float32
    f32r = mybir.dt.float32r
    nc = bacc.Bacc()

    NS = nbank * SPB
    lhs_d = nc.dram_tensor("lhs", [P, nbank * P], f32r, kind="ExternalInput")
    rhs_d = nc.dram_tensor("rhs", [P, nbank * BANKW], f32r, kind="ExternalInput")
    cm_d = nc.dram_tensor("cmask", [P, NS], f32, kind="ExternalInput")
    out_d = nc.dram_tensor("out", [1, 1], f32, kind="ExternalOutput")

    GB = nbank // 2   # banks per PSUM group (2 groups, bufs=2)

    with TileContext(nc) as tc:
        with tc.tile_pool(name="const", bufs=1) as cpool, \
             tc.tile_pool(name="psum", bufs=2, space="PSUM") as ppool:

            sp = mybir.EngineType.SP
            lhs_sb = cpool.tile_from(lhs_d[:, :], forced_dma_engine=sp)
            rhs_sb = cpool.tile_from(rhs_d[:, :], forced_dma_engine=sp)
            cm_sb = cpool.tile_from(cm_d[:, :], forced_dma_engine=sp)
            mnstrip = cpool.tile([P, NS], f32)
            nmxstrip = cpool.tile([P, NS], f32)
            v_t = cpool.tile([P, NS], f32)
            w_t = cpool.tile([P, NS], f32)
            g_t = cpool.tile([P, NS], f32)
            u_t = cpool.tile([P, NS], f32)
            vstrip = cpool.tile([P, NS], f32)
            ones_sb = cpool.tile([P, 1], f32)
            nc.vector.memset(ones_sb, 1.0)
            nc.vector.memset(nmxstrip, -BIG)

            def body():
                for grp in range(2):
                    ps = ppool.tile([P, GB * BANKW], f32, tag="ps")
                    for gb in range(GB):
                        b = grp * GB + gb
                        nc.tensor.matmul(
                            ps[:, gb * BANKW: gb * BANKW + SPB * H],
                            lhs_sb[0:3 * SPB, b * P:(b + 1) * P],
                            rhs_sb[0:3 * SPB,
                                   b * BANKW: b * BANKW + SPB * H],
                            start=True, stop=True,
                        )
                    for gb in range(GB):
                        b = grp * GB + gb
                        view = ps[:, gb * BANKW: gb * BANKW + SPB * H] \
                            .rearrange("p (s h) -> p s h", h=H)
                        so = b * SPB
                        nc.vector.tensor_reduce(
                            out=mnstrip[:, so:so + SPB], in_=view,
                            axis=mybir.AxisListType.X, op=mybir.AluOpType.min,
                        )
                        if b < mxbank:
                            nc.vector.tensor_reduce(
                                out=nmxstrip[:, so:so + SPB], in_=view,
                                axis=mybir.AxisListType.X,
                                op=mybir.AluOpType.max, negate=True,
                            )
                nc.vector.tensor_tensor(
                    out=v_t, in0=mnstrip, in1=nmxstrip, op=mybir.AluOpType.max)
                nc.scalar.activation(
                    out=w_t, in_=v_t, func=mybir.ActivationFunctionType.Sigmoid)
                nc.vector.tensor_scalar(
                    out=g_t, in0=v_t, scalar1=-float(EPS), scalar2=None,
                    op0=mybir.AluOpType.is_ge)
                nc.vector.tensor_tensor(
                    out=u_t, in0=w_t, in1=g_t, op=mybir.AluOpType.mult)
                nc.vector.tensor_tensor(
                    out=vstrip, in0=u_t, in1=cm_sb, op=mybir.AluOpType.mult)

            if loop is not None:
                import os as _os
                stg = _os.environ.get("LOOP_STAGGERED", "0") == "1"
                with tc.For_i(0, loop, 1, staggered_reset=stg) as _i:
                    body()
            else:
                for _ in range(reps):
                    body()

            acc = cpool.tile([P, 1], f32)
            nc.vector.tensor_reduce(
                out=acc, in_=vstrip, axis=mybir.AxisListType.X,
                op=mybir.AluOpType.add,
            )
            out_ps = ppool.tile([1, 1], f32, tag="ps2")
            nc.tensor.matmul(out_ps, acc, ones_sb, start=True, stop=True)
            out_sb = cpool.tile([1, 1], f32)
            nc.scalar.copy(out=out_sb, in_=out_ps)
            nc.sync.dma_start(out=out_d[:, :], in_=out_sb)

    nc.compile()
    return nc


def kernel(padded_clusters, padded_hulls, medoids, rotation_angles,
           translations, cluster_masks, hull_masks):
    pc = np.asarray(padded_clusters, dtype=np.float32)
    ph = np.asarray(padded_hulls, dtype=np.float32)
    med = np.asarray(medoids, dtype=np.float32)
    ang = np.asarray(rotation_angles, dtype=np.float32)
    tr = np.asarray(translations, dtype=np.float32)
    cm = np.asarray(cluster_masks)
    hm = np.asarray(hull_masks)

    cfg, in_maps = _plan_and_pack(pc, ph, med, ang, tr, cm, hm)

    key = ("nc", cfg[0], cfg[1])
    if key not in _NC_CACHE:
        _NC_CACHE[key] = _build_nc(cfg)
    nc = _NC_CACHE[key]

    from concourse.bass_utils import run_bass_kernel_spmd
    res = run_bass_kernel_spmd(nc, in_maps, core_ids=list(range(NCORES)))
    _NC_CACHE["last_results"] = res

    sep = sum(float(r["out"][0, 0]) for r in res.results) + cfg[2]
    total = (SEP_W * sep
             + T_PEN * float(np.sum(tr.astype(np.float64) ** 2))
             + R_PEN * float(np.sum(ang.astype(np.float64) ** 2)))
    return np.asarray(total, dtype=np.float32)


# revision 3
# speedup vs baseline: 1.3387x; 1.0003x over previous
"""Trainium2 Bass kernel for ClusterSeparationOptimizer (v2).

Math (identical to reference up to fp32 rounding):
  signed[i,n,j,h] = [x, y, 1] @ (A_i @ W[:, j, h])   (affine in the RAW point)
  mn = min_h signed, mx = max_h signed               (over valid edges)
  v  = max(mn, -mx)   -> v >= -EPS iff inside; v = min|signed| when inside
  viol = sigmoid(v) * (v >= -EPS) * cluster_mask
  out  = sum viol (i!=j, hull_ok) + 0.1*|translations|^2 + |angles|^2

Host-side planning (fp64, exact):
  * Only VALID points are packed: each cluster's n_i real points are
    kd-split into ceil(n_i/128) chunks of <=128; chunks padded to 128 with
    far sentinels (cmask=0, v<0 there by convexity).
  * Hull orientation is normalized (W flipped so interior => all s > 0).
  * Per (chunk, hull) pair, exact corner tests on the chunk bbox (signed is
    affine in the raw point; env_lo=min_h s is concave so its box-min is at
    a corner):
      - pruned   : some edge all-corners < -TAU and some all > TAU
                   -> every point sign-mixed -> viol == 0.
      - deep     : env_lo >= DEEP at all corners -> sigmoid(mn) = 1 within
                   e^-DEEP per point; host adds count*1.0, pair skipped.
      - interior : env_lo >= POSM at all corners -> mx > 0 > -mx <= mn, so
                   v = mn exactly; the device skips the max-reduce.
      - boundary : both reduces.

Device (SPMD one program, per-core data):
  Pairs are packed as 40-wide column slots, 12 per PSUM bank.  One
  float32r matmul per bank: lhsT[K<=36,128] holds [x,y,1] of each slot's
  chunk K-triple (block-diagonal rhs holds each slot's 40 G columns), so a
  single wide (480-col, >=256 => 1 cycle/row) matmul computes 12 slots'
  signed distances for 128 points.  Banks are processed in 2 groups of 3
  with a bufs=2 PSUM pool so group g+1 matmuls overlap group g reduces.
  Per bank: DVE tensor_reduce(min) -> mn strip; for the first mxbank banks
  (boundary slots first) tensor_reduce(max, negate) -> -mx strip; the
  interior tail of the -mx strip is pre-set to -BIG once.  Tail:
  v = max(mn, nmx); sigmoid on ACT; (v >= -EPS) gate; * cmask -> vstrip.
  Final: reduce_sum + ones-matmul -> scalar; host all-reduces the 8 cores
  and adds the deep-interior count and penalty terms.
"""

import math

import numpy as np

C, N, H = 24, 1536, 40
NCORES = 8
P = 128                    # points per chunk / partition dim
SPB = 12                   # slots per 512-col PSUM bank (12*40=480)
BANKW = 512
SEP_W, T_PEN, R_PEN = 1.0, 0.1, 1.0
EPS = 1e-8
BIG = 1e30
TAU = 1e-5                 # prune margin
POSM = 1e-2                # interior margin (device fp32 slop ~1e-4)
DEEP = 8.5                 # deep-interior skip: per-point err <= e^-8.5
SENT = 1.0e6               # sentinel coordinate for padded points

_NC_CACHE = {}


def _transform64(x, med, ang, tr):
    c, s = np.cos(ang), np.sin(ang)
    xc = x[..., 0] - med[:, None, 0]
    yc = x[..., 1] - med[:, None, 1]
    px = c[:, None] * xc - s[:, None] * yc + (med[:, 0] + tr[:, 0])[:, None]
    py = s[:, None] * xc + c[:, None] * yc + (med[:, 1] + tr[:, 1])[:, None]
    return np.stack([px, py], -1)


def _host_coeffs(ph, med, ang, tr, hm):
    """G[i] = A_i @ W: (C, 3, C, H) float64; rows act on raw [x, y, 1].

    W is orientation-normalized so that hull interiors have s > 0."""
    hulT = _transform64(ph, med, ang, tr)
    hx, hy = hulT[..., 0], hulT[..., 1]
    ex = np.roll(hx, -1, axis=1) - hx
    ey = np.roll(hy, -1, axis=1) - hy
    elen_raw = np.sqrt(ex * ex + ey * ey)
    elen = elen_raw + EPS
    evalid = elen_raw > 1e-6
    a = ex / elen
    b = -ey / elen
    d = -(ex * hy - ey * hx) / elen

    W = np.stack([b, a, d], axis=0)  # (3, C, H): coeffs on transformed [x,y,1]
    degenerate = np.zeros(C, bool)
    flip = np.ones(C)
    for j in range(C):
        inv = ~evalid[j]
        val = np.nonzero(evalid[j])[0]
        if inv.any():
            if len(val) > 0:
                W[:, j, inv] = W[:, j, val[-1]][:, None]
            else:
                W[:, j, :] = np.array([0.0, 0.0, BIG])[:, None]
                degenerate[j] = True
        if not degenerate[j]:
            vm = hm[j] if hm[j].any() else np.ones(H, bool)
            cx, cy = hulT[j, vm, 0].mean(), hulT[j, vm, 1].mean()
            sc = W[0, j, val] * cx + W[1, j, val] * cy + W[2, j, val]
            if np.median(sc) < 0:
                flip[j] = -1.0
                W[:, j, :] = -W[:, j, :]

    c, s = np.cos(ang), np.sin(ang)
    A = np.zeros((C, 3, 3))
    A[:, 0, 0] = c
    A[:, 0, 1] = s
    A[:, 1, 0] = -s
    A[:, 1, 1] = c
    A[:, 2, 0] = med[:, 0] + tr[:, 0] - c * med[:, 0] + s * med[:, 1]
    A[:, 2, 1] = med[:, 1] + tr[:, 1] - s * med[:, 0] - c * med[:, 1]
    A[:, 2, 2] = 1.0

    G = np.einsum("ikl,lm->ikm", A, W.reshape(3, C * H))
    return G.reshape(C, 3, C, H), hulT, degenerate


def _kd_split(p, ids, parts):
    """Split index array ids into `parts` groups of near-equal size (each
    <= ceil(len/parts)) by recursive median cuts on the wider dimension."""
    if parts == 1:
        return [ids]
    q = p[ids]
    dim = 0 if np.ptp(q[:, 0]) >= np.ptp(q[:, 1]) else 1
    order = ids[np.argsort(q[:, dim], kind="stable")]
    pl = parts // 2
    k = (len(order) * pl + parts - 1) // parts
    return _kd_split(p, order[:k], pl) + _kd_split(p, order[k:], parts - pl)


def _plan_and_pack(pc, ph, med, ang, tr, cm, hm):
    """Returns (cfg, in_maps): cfg=(nbank, mxbank, host_extra)."""
    med64 = med.astype(np.float64)
    ang64 = ang.astype(np.float64)
    tr64 = tr.astype(np.float64)
    G, hulT, degen = _host_coeffs(ph.astype(np.float64), med64, ang64, tr64, hm)
    hull_ok = hm.sum(-1) >= 3

    host_deep = 0.0
    boundary = []   # (i, chunk_pts_idx (np array of <=128), j)
    interior = []
    for i in range(C):
        valid = np.nonzero(cm[i])[0]
        if len(valid) == 0:
            continue
        parts = (len(valid) + P - 1) // P
        Gi = G[i].reshape(3, C * H)
        for ch in _kd_split(pc[i].astype(np.float64), valid, parts):
            q = pc[i, ch].astype(np.float64)
            qmin, qmax = q.min(0), q.max(0)
            corners = np.array(
                [[qmin[0], qmin[1], 1.0], [qmin[0], qmax[1], 1.0],
                 [qmax[0], qmin[1], 1.0], [qmax[0], qmax[1], 1.0]])
            sc = (corners @ Gi).reshape(4, C, H)
            neg_edge = (sc.max(0) < -TAU).any(-1)
            pos_edge = (sc.min(0) > TAU).any(-1)
            prunable = neg_edge & pos_edge
            env_lo_min = sc.min(-1).min(0)          # (C,) box-min of min_h s
            for j in range(C):
                if j == i or not hull_ok[j]:
                    continue
                if not degen[j] and prunable[j]:
                    continue
                if degen[j] or env_lo_min[j] >= DEEP:
                    host_deep += float(len(ch))
                    continue
                if env_lo_min[j] >= POSM:
                    interior.append((i, ch, j))
                else:
                    boundary.append((i, ch, j))

    per_b = [boundary[c::NCORES] for c in range(NCORES)]
    per_i = [interior[c::NCORES] for c in range(NCORES)]
    max_b = max(len(x) for x in per_b)
    max_s = max(len(b) + len(t) for b, t in zip(per_b, per_i))
    mxbank = (max_b + SPB - 1) // SPB
    nbank = max((max_s + SPB - 1) // SPB, mxbank)
    nbank += nbank % 2        # even # banks -> two equal groups
    assert nbank <= 6, f"PSUM budget exceeded: nbank={nbank}"

    in_maps = []
    for c in range(NCORES):
        slots = per_b[c] + per_i[c]
        lhs = np.zeros((P, nbank * P), np.float32)
        rhs = np.zeros((P, nbank * BANKW), np.float32)
        cm3 = np.zeros((P, nbank * SPB), np.float32)
        for b in range(nbank):
            tri = {}
            for si, (i, ch, j) in enumerate(slots[b * SPB:(b + 1) * SPB]):
                key = (i, ch.tobytes())
                if key not in tri:
                    t = tri[key] = len(tri)
                    n = len(ch)
                    lhs[3 * t + 0, b * P: b * P + n] = pc[i, ch, 0]
                    lhs[3 * t + 1, b * P: b * P + n] = pc[i, ch, 1]
                    lhs[3 * t + 0, b * P + n:(b + 1) * P] = SENT
                    lhs[3 * t + 1, b * P + n:(b + 1) * P] = SENT
                    lhs[3 * t + 2, b * P:(b + 1) * P] = 1.0
                t = tri[key]
                co = b * BANKW + si * H
                rhs[3 * t: 3 * t + 3, co: co + H] = G[i, :, j, :]
                cm3[: len(ch), b * SPB + si] = 1.0
        in_maps.append({
            "lhs": np.ascontiguousarray(lhs),
            "rhs": np.ascontiguousarray(rhs),
            "cmask": np.ascontiguousarray(cm3),
        })
    return (nbank, mxbank, host_deep), in_maps


def _build_nc(cfg, reps=1, loop=None):
    import concourse.bacc as bacc
    import concourse.mybir as mybir
    from concourse.tile import TileContext

    nbank, mxbank = cfg[0], cfg[1]
    f32 = mybir.dt.float32
    f32r = mybir.dt.float32r
    nc = bacc.Bacc()

    NS = nbank * SPB
    lhs_d = nc.dram_tensor("lhs", [P, nbank * P], f32r, kind="ExternalInput")
    rhs_d = nc.dram_tensor("rhs", [P, nbank * BANKW], f32r, kind="ExternalInput")
    cm_d = nc.dram_tensor("cmask", [P, NS], f32, kind="ExternalInput")
    out_d = nc.dram_tensor("out", [1, 1], f32, kind="ExternalOutput")

    GB = nbank // 2   # banks per PSUM group (2 groups, bufs=2)

    with TileContext(nc) as tc:
        with tc.tile_pool(name="const", bufs=1) as cpool, \
             tc.tile_pool(name="psum", bufs=2, space="PSUM") as ppool:

            sp = mybir.EngineType.SP
            lhs_sb = cpool.tile_from(lhs_d[:, :], forced_dma_engine=sp)
            rhs_sb = cpool.tile_from(rhs_d[:, :], forced_dma_engine=sp)
            cm_sb = cpool.tile_from(cm_d[:, :], forced_dma_engine=sp)
            mnstrip = cpool.tile([P, NS], f32)
            nmxstrip = cpool.tile([P, NS], f32)
            v_t = cpool.tile([P, NS], f32)
            w_t = cpool.tile([P, NS], f32)
            g_t = cpool.tile([P, NS], f32)
            u_t = cpool.tile([P, NS], f32)
            vstrip = cpool.tile([P, NS], f32)
            ones_sb = cpool.tile([P, 1], f32)
            nc.vector.memset(ones_sb, 1.0)
            nc.vector.memset(nmxstrip, -BIG)

            def body():
                for grp in range(2):
                    ps = ppool.tile([P, GB * BANKW], f32, tag="ps")
                    for gb in range(GB):
                        b = grp * GB + gb
                        nc.tensor.matmul(
                            ps[:, gb * BANKW: gb * BANKW + SPB * H],
                            lhs_sb[0:3 * SPB, b * P:(b + 1) * P],
                            rhs_sb[0:3 * SPB,
                                   b * BANKW: b * BANKW + SPB * H],
                            start=True, stop=True,
                        )
                    # one 4D-view reduce per op covering the whole group
                    view = ps.rearrange("p (b k) -> p b k", b=GB)[:, :, 0:SPB * H] \
                        .rearrange("p b (s h) -> p b s h", h=H)
                    so = grp * GB * SPB
                    sw = GB * SPB
                    nc.vector.tensor_reduce(
                        out=mnstrip[:, so:so + sw], in_=view,
                        axis=mybir.AxisListType.X, op=mybir.AluOpType.min,
                    )
                    mxb = min(mxbank - grp * GB, GB)   # banks needing max
                    if mxb > 0:
                        mview = view if mxb == GB else \
                            ps.rearrange("p (b k) -> p b k", b=GB)[:, 0:mxb, 0:SPB * H] \
                              .rearrange("p b (s h) -> p b s h", h=H)
                        nc.vector.tensor_reduce(
                            out=nmxstrip[:, so:so + mxb * SPB], in_=mview,
                            axis=mybir.AxisListType.X,
                            op=mybir.AluOpType.max, negate=True,
                        )
                nc.vector.tensor_tensor(
                    out=v_t, in0=mnstrip, in1=nmxstrip, op=mybir.AluOpType.max)
                nc.scalar.activation(
                    out=w_t, in_=v_t, func=mybir.ActivationFunctionType.Sigmoid)
                nc.vector.tensor_scalar(
                    out=g_t, in0=v_t, scalar1=-float(EPS), scalar2=None,
                    op0=mybir.AluOpType.is_ge)
                nc.gpsimd.tensor_tensor(
                    out=u_t, in0=w_t, in1=g_t, op=mybir.AluOpType.mult)
                nc.gpsimd.tensor_tensor(
                    out=vstrip, in0=u_t, in1=cm_sb, op=mybir.AluOpType.mult)

            if loop is not None:
                import os as _os
                stg = _os.environ.get("LOOP_STAGGERED", "0") == "1"
                with tc.For_i(0, loop, 1, staggered_reset=stg) as _i:
                    body()
            else:
                for _ in range(reps):
                    body()

            acc = cpool.tile([P, 1], f32)
            nc.vector.tensor_reduce(
                out=acc, in_=vstrip, axis=mybir.AxisListType.X,
                op=mybir.AluOpType.add,
            )
            out_ps = ppool.tile([1, 1], f32, tag="ps2")
            nc.tensor.matmul(out_ps, acc, ones_sb, start=True, stop=True)
            out_sb = cpool.tile([1, 1], f32)
            nc.scalar.copy(out=out_sb, in_=out_ps)
            nc.sync.dma_start(out=out_d[:, :], in_=out_sb)

    nc.compile()
    return nc


def kernel(padded_clusters, padded_hulls, medoids, rotation_angles,
           translations, cluster_masks, hull_masks):
    pc = np.asarray(padded_clusters, dtype=np.float32)
    ph = np.asarray(padded_hulls, dtype=np.float32)
    med = np.asarray(medoids, dtype=np.float32)
    ang = np.asarray(rotation_angles, dtype=np.float32)
    tr = np.asarray(translations, dtype=np.float32)
    cm = np.asarray(cluster_masks)
    hm = np.asarray(hull_masks)

    cfg, in_maps = _plan_and_pack(pc, ph, med, ang, tr, cm, hm)

    key = ("nc", cfg[0], cfg[1])
    if key not in _NC_CACHE:
        _NC_CACHE[key] = _build_nc(cfg)
    nc = _NC_CACHE[key]

    from concourse.bass_utils import run_bass_kernel_spmd
    res = run_bass_kernel_spmd(nc, in_maps, core_ids=list(range(NCORES)))
    _NC_CACHE["last_results"] = res

    sep = sum(float(r["out"][0, 0]) for r in res.results) + cfg[2]
    total = (SEP_W * sep
             + T_PEN * float(np.sum(tr.astype(np.float64) ** 2))
             + R_PEN * float(np.sum(ang.astype(np.float64) ** 2)))
    return np.asarray(total, dtype=np.float32)


# revision 9
# speedup vs baseline: 2.0673x; 1.5443x over previous
"""Trainium2 Bass kernel for ClusterSeparationOptimizer (v2).

Math (identical to reference up to fp32 rounding):
  signed[i,n,j,h] = [x, y, 1] @ (A_i @ W[:, j, h])   (affine in the RAW point)
  mn = min_h signed, mx = max_h signed               (over valid edges)
  v  = max(mn, -mx)   -> v >= -EPS iff inside; v = min|signed| when inside
  viol = sigmoid(v) * (v >= -EPS) * cluster_mask
  out  = sum viol (i!=j, hull_ok) + 0.1*|translations|^2 + |angles|^2

Host-side planning (fp64, exact):
  * Only VALID points are packed: each cluster's n_i real points are
    kd-split into ceil(n_i/128) chunks of <=128; chunks padded to 128 with
    far sentinels (cmask=0, v<0 there by convexity).
  * Hull orientation is normalized (W flipped so interior => all s > 0).
  * Per (chunk, hull) pair, exact corner tests on the chunk bbox (signed is
    affine in the raw point; env_lo=min_h s is concave so its box-min is at
    a corner):
      - pruned   : some edge all-corners < -TAU and some all > TAU
                   -> every point sign-mixed -> viol == 0.
      - deep     : env_lo >= DEEP at all corners -> sigmoid(mn) = 1 within
                   e^-DEEP per point; host adds count*1.0, pair skipped.
      - interior : env_lo >= POSM at all corners -> mx > 0 > -mx <= mn, so
                   v = mn exactly; the device skips the max-reduce.
      - boundary : both reduces.

Device (SPMD one program, per-core data):
  Pairs are packed as 40-wide column slots, 12 per PSUM bank.  One
  float32r matmul per bank: lhsT[K<=36,128] holds [x,y,1] of each slot's
  chunk K-triple (block-diagonal rhs holds each slot's 40 G columns), so a
  single wide (480-col, >=256 => 1 cycle/row) matmul computes 12 slots'
  signed distances for 128 points.  Banks are processed in 2 groups of 3
  with a bufs=2 PSUM pool so group g+1 matmuls overlap group g reduces.
  Per bank: DVE tensor_reduce(min) -> mn strip; for the first mxbank banks
  (boundary slots first) tensor_reduce(max, negate) -> -mx strip; the
  interior tail of the -mx strip is pre-set to -BIG once.  Tail:
  v = max(mn, nmx); sigmoid on ACT; (v >= -EPS) gate; * cmask -> vstrip.
  Final: reduce_sum + ones-matmul -> scalar; host all-reduces the 8 cores
  and adds the deep-interior count and penalty terms.
"""

import numpy as np

C, N, H = 24, 1536, 40
NCORES = 8
P = 128                    # points per chunk / partition dim
SPB = 12                   # slots per 512-col PSUM bank (12*40=480)
BANKW = 512
SEP_W, T_PEN, R_PEN = 1.0, 0.1, 1.0
EPS = 1e-8
BIG = 1e30
TAU = 1e-5                 # prune margin
POSM = 1e-2                # interior margin (device fp32 slop ~1e-4)
DEEP = 8.5                 # deep-interior skip: per-point err <= e^-8.5
SENT = 1.0e6               # sentinel coordinate for padded points
UNROLL = 2                 # bodies per For_i iteration (timing loop only)

_NC_CACHE = {}


def _transform64(x, med, ang, tr):
    c, s = np.cos(ang), np.sin(ang)
    xc = x[..., 0] - med[:, None, 0]
    yc = x[..., 1] - med[:, None, 1]
    px = c[:, None] * xc - s[:, None] * yc + (med[:, 0] + tr[:, 0])[:, None]
    py = s[:, None] * xc + c[:, None] * yc + (med[:, 1] + tr[:, 1])[:, None]
    return np.stack([px, py], -1)


def _host_coeffs(ph, med, ang, tr, hm):
    """G[i] = A_i @ W: (C, 3, C, H) float64; rows act on raw [x, y, 1].

    W is orientation-normalized so that hull interiors have s > 0."""
    hulT = _transform64(ph, med, ang, tr)
    hx, hy = hulT[..., 0], hulT[..., 1]
    ex = np.roll(hx, -1, axis=1) - hx
    ey = np.roll(hy, -1, axis=1) - hy
    elen_raw = np.sqrt(ex * ex + ey * ey)
    elen = elen_raw + EPS
    evalid = elen_raw > 1e-6
    a = ex / elen
    b = -ey / elen
    d = -(ex * hy - ey * hx) / elen

    W = np.stack([b, a, d], axis=0)  # (3, C, H): coeffs on transformed [x,y,1]
    degenerate = np.zeros(C, bool)
    flip = np.ones(C)
    for j in range(C):
        inv = ~evalid[j]
        val = np.nonzero(evalid[j])[0]
        if inv.any():
            if len(val) > 0:
                W[:, j, inv] = W[:, j, val[-1]][:, None]
            else:
                W[:, j, :] = np.array([0.0, 0.0, BIG])[:, None]
                degenerate[j] = True
        if not degenerate[j]:
            vm = hm[j] if hm[j].any() else np.ones(H, bool)
            cx, cy = hulT[j, vm, 0].mean(), hulT[j, vm, 1].mean()
            sc = W[0, j, val] * cx + W[1, j, val] * cy + W[2, j, val]
            if np.median(sc) < 0:
                flip[j] = -1.0
                W[:, j, :] = -W[:, j, :]

    c, s = np.cos(ang), np.sin(ang)
    A = np.zeros((C, 3, 3))
    A[:, 0, 0] = c
    A[:, 0, 1] = s
    A[:, 1, 0] = -s
    A[:, 1, 1] = c
    A[:, 2, 0] = med[:, 0] + tr[:, 0] - c * med[:, 0] + s * med[:, 1]
    A[:, 2, 1] = med[:, 1] + tr[:, 1] - s * med[:, 0] - c * med[:, 1]
    A[:, 2, 2] = 1.0

    G = np.einsum("ikl,lm->ikm", A, W.reshape(3, C * H))
    return G.reshape(C, 3, C, H), hulT, degenerate


def _kd_split(p, ids, parts):
    """Split index array ids into `parts` groups of near-equal size (each
    <= ceil(len/parts)) by recursive median cuts on the wider dimension."""
    if parts == 1:
        return [ids]
    q = p[ids]
    dim = 0 if np.ptp(q[:, 0]) >= np.ptp(q[:, 1]) else 1
    order = ids[np.argsort(q[:, dim], kind="stable")]
    pl = parts // 2
    k = (len(order) * pl + parts - 1) // parts
    return _kd_split(p, order[:k], pl) + _kd_split(p, order[k:], parts - pl)


def _plan_and_pack(pc, ph, med, ang, tr, cm, hm):
    """Returns (cfg, in_maps): cfg=(nbank, mxbank, host_extra)."""
    med64 = med.astype(np.float64)
    ang64 = ang.astype(np.float64)
    tr64 = tr.astype(np.float64)
    G, hulT, degen = _host_coeffs(ph.astype(np.float64), med64, ang64, tr64, hm)
    hull_ok = hm.sum(-1) >= 3

    host_deep = 0.0
    boundary = []   # (i, chunk_pts_idx (np array of <=128), j)
    interior = []
    for i in range(C):
        valid = np.nonzero(cm[i])[0]
        if len(valid) == 0:
            continue
        parts = (len(valid) + P - 1) // P
        Gi = G[i].reshape(3, C * H)
        for ch in _kd_split(pc[i].astype(np.float64), valid, parts):
            q = pc[i, ch].astype(np.float64)
            qmin, qmax = q.min(0), q.max(0)
            corners = np.array(
                [[qmin[0], qmin[1], 1.0], [qmin[0], qmax[1], 1.0],
                 [qmax[0], qmin[1], 1.0], [qmax[0], qmax[1], 1.0]])
            sc = (corners @ Gi).reshape(4, C, H)
            neg_edge = (sc.max(0) < -TAU).any(-1)
            pos_edge = (sc.min(0) > TAU).any(-1)
            prunable = neg_edge & pos_edge
            env_lo_min = sc.min(-1).min(0)          # (C,) box-min of min_h s
            for j in range(C):
                if j == i or not hull_ok[j]:
                    continue
                if not degen[j] and prunable[j]:
                    continue
                if degen[j] or env_lo_min[j] >= DEEP:
                    host_deep += float(len(ch))
                    continue
                if env_lo_min[j] >= POSM:
                    interior.append((i, ch, j))
                else:
                    boundary.append((i, ch, j))

    per_b = [boundary[c::NCORES] for c in range(NCORES)]
    per_i = [interior[c::NCORES] for c in range(NCORES)]
    max_b = max(len(x) for x in per_b)
    max_s = max(len(b) + len(t) for b, t in zip(per_b, per_i))
    mxbank = (max_b + SPB - 1) // SPB
    nbank = max((max_s + SPB - 1) // SPB, mxbank)
    nbank += nbank % 2        # even # banks -> two equal groups
    assert nbank <= 6, f"PSUM budget exceeded: nbank={nbank}"

    in_maps = []
    for c in range(NCORES):
        slots = per_b[c] + per_i[c]
        lhs = np.zeros((P, nbank * P), np.float32)
        rhs = np.zeros((P, nbank * BANKW), np.float32)
        cm3 = np.zeros((P, nbank * SPB), np.float32)
        for b in range(nbank):
            tri = {}
            for si, (i, ch, j) in enumerate(slots[b * SPB:(b + 1) * SPB]):
                key = (i, ch.tobytes())
                if key not in tri:
                    t = tri[key] = len(tri)
                    n = len(ch)
                    lhs[3 * t + 0, b * P: b * P + n] = pc[i, ch, 0]
                    lhs[3 * t + 1, b * P: b * P + n] = pc[i, ch, 1]
                    lhs[3 * t + 0, b * P + n:(b + 1) * P] = SENT
                    lhs[3 * t + 1, b * P + n:(b + 1) * P] = SENT
                    lhs[3 * t + 2, b * P:(b + 1) * P] = 1.0
                t = tri[key]
                co = b * BANKW + si * H
                rhs[3 * t: 3 * t + 3, co: co + H] = G[i, :, j, :]
                cm3[: len(ch), b * SPB + si] = 1.0
        in_maps.append({
            "lhs": np.ascontiguousarray(lhs),
            "rhs": np.ascontiguousarray(rhs),
            "cmask": np.ascontiguousarray(cm3),
        })
    return (nbank, mxbank, host_deep), in_maps


def _build_nc(cfg, reps=1, loop=None):
    import concourse.bacc as bacc
    import concourse.mybir as mybir
    from concourse.tile import TileContext

    nbank, mxbank = cfg[0], cfg[1]
    f32 = mybir.dt.float32
    f32r = mybir.dt.float32r
    nc = bacc.Bacc()

    NS = nbank * SPB
    lhs_d = nc.dram_tensor("lhs", [P, nbank * P], f32r, kind="ExternalInput")
    rhs_d = nc.dram_tensor("rhs", [P, nbank * BANKW], f32r, kind="ExternalInput")
    cm_d = nc.dram_tensor("cmask", [P, NS], f32, kind="ExternalInput")
    out_d = nc.dram_tensor("out", [1, 1], f32, kind="ExternalOutput")

    GB = nbank // 2   # banks per PSUM group (2 groups, bufs=2)

    import os as _os
    unroll = int(_os.environ.get("UNROLL", str(UNROLL))) if loop is not None else 1

    with TileContext(nc) as tc:
        with tc.tile_pool(name="const", bufs=1) as cpool, \
             tc.tile_pool(name="work", bufs=2) as wpool, \
             tc.tile_pool(name="psum", bufs=2, space="PSUM") as ppool:

            sp = mybir.EngineType.SP
            lhs_sb = cpool.tile_from(lhs_d[:, :], forced_dma_engine=sp)
            rhs_sb = cpool.tile_from(rhs_d[:, :], forced_dma_engine=sp)
            cm_sb = cpool.tile_from(cm_d[:, :], forced_dma_engine=sp)
            vstrip = cpool.tile([P, NS], f32)
            ones_sb = cpool.tile([P, 1], f32)
            nc.vector.memset(ones_sb, 1.0)

            def body():
                mnstrip = wpool.tile([P, NS], f32, tag="mn")
                nmxstrip = wpool.tile([P, NS], f32, tag="nmx")
                v_t = wpool.tile([P, NS], f32, tag="v")
                w_t = wpool.tile([P, NS], f32, tag="w")
                gm_t = wpool.tile([P, NS], f32, tag="gm")
                for grp in range(2):
                    ps = ppool.tile([P, GB * BANKW], f32, tag="ps")
                    for gb in range(GB):
                        b = grp * GB + gb
                        nc.tensor.matmul(
                            ps[:, gb * BANKW: gb * BANKW + SPB * H],
                            lhs_sb[0:3 * SPB, b * P:(b + 1) * P],
                            rhs_sb[0:3 * SPB,
                                   b * BANKW: b * BANKW + SPB * H],
                            start=True, stop=True,
                        )
                    # one 4D-view reduce per op covering the whole group
                    view = ps.rearrange("p (b k) -> p b k", b=GB)[:, :, 0:SPB * H] \
                        .rearrange("p b (s h) -> p b s h", h=H)
                    so = grp * GB * SPB
                    sw = GB * SPB
                    nc.vector.tensor_reduce(
                        out=mnstrip[:, so:so + sw], in_=view,
                        axis=mybir.AxisListType.X, op=mybir.AluOpType.min,
                    )
                    mxb = min(mxbank - grp * GB, GB)   # banks needing max
                    if mxb > 0:
                        mview = view if mxb == GB else \
                            ps.rearrange("p (b k) -> p b k", b=GB)[:, 0:mxb, 0:SPB * H] \
                              .rearrange("p b (s h) -> p b s h", h=H)
                        nc.vector.tensor_reduce(
                            out=nmxstrip[:, so:so + mxb * SPB], in_=mview,
                            axis=mybir.AxisListType.X,
                            op=mybir.AluOpType.max, negate=True,
                        )
                # v = max(mn, -mx); for mx-less slots v = mn
                MXS = mxbank * SPB
                nc.vector.tensor_tensor(
                    out=v_t[:, 0:MXS], in0=mnstrip[:, 0:MXS],
                    in1=nmxstrip[:, 0:MXS], op=mybir.AluOpType.max)
                if MXS < NS:
                    nc.scalar.copy(out=v_t[:, MXS:NS], in_=mnstrip[:, MXS:NS])
                # sigmoid on ACT in parallel with the DVE gate*cmask chain
                nc.scalar.activation(
                    out=w_t, in_=v_t, func=mybir.ActivationFunctionType.Sigmoid)
                nc.vector.tensor_scalar(
                    out=gm_t, in0=v_t, scalar1=-float(EPS), scalar2=None,
                    op0=mybir.AluOpType.is_ge)
                nc.gpsimd.tensor_tensor(
                    out=gm_t, in0=gm_t, in1=cm_sb, op=mybir.AluOpType.mult)
                nc.gpsimd.tensor_tensor(
                    out=vstrip, in0=w_t, in1=gm_t, op=mybir.AluOpType.mult)

            if loop is not None:
                stg = _os.environ.get("LOOP_STAGGERED", "0") == "1"
                with tc.For_i(0, loop, 1, staggered_reset=stg) as _i:
                    for _ in range(unroll):
                        body()
            else:
                for _ in range(reps):
                    body()

            acc = cpool.tile([P, 1], f32)
            nc.vector.tensor_reduce(
                out=acc, in_=vstrip, axis=mybir.AxisListType.X,
                op=mybir.AluOpType.add,
            )
            out_ps = ppool.tile([1, 1], f32, tag="ps2")
            nc.tensor.matmul(out_ps, acc, ones_sb, start=True, stop=True)
            out_sb = cpool.tile([1, 1], f32)
            nc.scalar.copy(out=out_sb, in_=out_ps)
            nc.sync.dma_start(out=out_d[:, :], in_=out_sb)

    nc.compile()
    return nc


def kernel(padded_clusters, padded_hulls, medoids, rotation_angles,
           translations, cluster_masks, hull_masks):
    pc = np.asarray(padded_clusters, dtype=np.float32)
    ph = np.asarray(padded_hulls, dtype=np.float32)
    med = np.asarray(medoids, dtype=np.float32)
    ang = np.asarray(rotation_angles, dtype=np.float32)
    tr = np.asarray(translations, dtype=np.float32)
    cm = np.asarray(cluster_masks)
    hm = np.asarray(hull_masks)

    cfg, in_maps = _plan_and_pack(pc, ph, med, ang, tr, cm, hm)

    key = ("nc", cfg[0], cfg[1])
    if key not in _NC_CACHE:
        _NC_CACHE[key] = _build_nc(cfg)
    nc = _NC_CACHE[key]

    from concourse.bass_utils import run_bass_kernel_spmd
    res = run_bass_kernel_spmd(nc, in_maps, core_ids=list(range(NCORES)))
    _NC_CACHE["last_results"] = res

    sep = sum(float(r["out"][0, 0]) for r in res.results) + cfg[2]
    total = (SEP_W * sep
             + T_PEN * float(np.sum(tr.astype(np.float64) ** 2))
             + R_PEN * float(np.sum(ang.astype(np.float64) ** 2)))
    return np.asarray(total, dtype=np.float32)


# revision 10
# speedup vs baseline: 2.5384x; 1.2279x over previous
"""Trainium2 Bass kernel for ClusterSeparationOptimizer (v2).

Math (identical to reference up to fp32 rounding):
  signed[i,n,j,h] = [x, y, 1] @ (A_i @ W[:, j, h])   (affine in the RAW point)
  mn = min_h signed, mx = max_h signed               (over valid edges)
  v  = max(mn, -mx)   -> v >= -EPS iff inside; v = min|signed| when inside
  viol = sigmoid(v) * (v >= -EPS) * cluster_mask
  out  = sum viol (i!=j, hull_ok) + 0.1*|translations|^2 + |angles|^2

Host-side planning (fp64, exact):
  * Only VALID points are packed: each cluster's n_i real points are
    kd-split into ceil(n_i/128) chunks of <=128; chunks padded to 128 with
    far sentinels (cmask=0, v<0 there by convexity).
  * Hull orientation is normalized (W flipped so interior => all s > 0).
  * Per (chunk, hull) pair, exact corner tests on the chunk bbox (signed is
    affine in the raw point; env_lo=min_h s is concave so its box-min is at
    a corner):
      - pruned   : some edge all-corners < -TAU and some all > TAU
                   -> every point sign-mixed -> viol == 0.
      - deep     : env_lo >= DEEP at all corners -> sigmoid(mn) = 1 within
                   e^-DEEP per point; host adds count*1.0, pair skipped.
      - interior : env_lo >= POSM at all corners -> mx > 0 > -mx <= mn, so
                   v = mn exactly; the device skips the max-reduce.
      - boundary : both reduces.

Device (SPMD one program, per-core data):
  Pairs are packed as 40-wide column slots, 12 per PSUM bank.  One
  float32r matmul per bank: lhsT[K<=36,128] holds [x,y,1] of each slot's
  chunk K-triple (block-diagonal rhs holds each slot's 40 G columns), so a
  single wide (480-col, >=256 => 1 cycle/row) matmul computes 12 slots'
  signed distances for 128 points.  Banks are processed in 2 groups of 3
  with a bufs=2 PSUM pool so group g+1 matmuls overlap group g reduces.
  Per bank: DVE tensor_reduce(min) -> mn strip; for the first mxbank banks
  (boundary slots first) tensor_reduce(max, negate) -> -mx strip; the
  interior tail of the -mx strip is pre-set to -BIG once.  Tail:
  v = max(mn, nmx); sigmoid on ACT; (v >= -EPS) gate; * cmask -> vstrip.
  Final: reduce_sum + ones-matmul -> scalar; host all-reduces the 8 cores
  and adds the deep-interior count and penalty terms.
"""

import numpy as np

C, N, H = 24, 1536, 40
NCORES = 8
P = 128                    # points per chunk / partition dim
SPB = 12                   # slots per 512-col PSUM bank (12*40=480)
BANKW = 512
SEP_W, T_PEN, R_PEN = 1.0, 0.1, 1.0
EPS = 1e-8
BIG = 1e30
TAU = 1e-5                 # prune margin
POSM = 1e-2                # interior margin (device fp32 slop ~1e-4)
DEEP = 8.5                 # deep-interior skip: per-point err <= e^-8.5
SENT = 1.0e6               # sentinel coordinate for padded points
UNROLL = 4                 # bodies per For_i iteration (timing loop only)

_NC_CACHE = {}


def _transform64(x, med, ang, tr):
    c, s = np.cos(ang), np.sin(ang)
    xc = x[..., 0] - med[:, None, 0]
    yc = x[..., 1] - med[:, None, 1]
    px = c[:, None] * xc - s[:, None] * yc + (med[:, 0] + tr[:, 0])[:, None]
    py = s[:, None] * xc + c[:, None] * yc + (med[:, 1] + tr[:, 1])[:, None]
    return np.stack([px, py], -1)


def _host_coeffs(ph, med, ang, tr, hm):
    """G[i] = A_i @ W: (C, 3, C, H) float64; rows act on raw [x, y, 1].

    W is orientation-normalized so that hull interiors have s > 0."""
    hulT = _transform64(ph, med, ang, tr)
    hx, hy = hulT[..., 0], hulT[..., 1]
    ex = np.roll(hx, -1, axis=1) - hx
    ey = np.roll(hy, -1, axis=1) - hy
    elen_raw = np.sqrt(ex * ex + ey * ey)
    elen = elen_raw + EPS
    evalid = elen_raw > 1e-6
    a = ex / elen
    b = -ey / elen
    d = -(ex * hy - ey * hx) / elen

    W = np.stack([b, a, d], axis=0)  # (3, C, H): coeffs on transformed [x,y,1]
    degenerate = np.zeros(C, bool)
    flip = np.ones(C)
    for j in range(C):
        inv = ~evalid[j]
        val = np.nonzero(evalid[j])[0]
        if inv.any():
            if len(val) > 0:
                W[:, j, inv] = W[:, j, val[-1]][:, None]
            else:
                W[:, j, :] = np.array([0.0, 0.0, BIG])[:, None]
                degenerate[j] = True
        if not degenerate[j]:
            vm = hm[j] if hm[j].any() else np.ones(H, bool)
            cx, cy = hulT[j, vm, 0].mean(), hulT[j, vm, 1].mean()
            sc = W[0, j, val] * cx + W[1, j, val] * cy + W[2, j, val]
            if np.median(sc) < 0:
                flip[j] = -1.0
                W[:, j, :] = -W[:, j, :]

    c, s = np.cos(ang), np.sin(ang)
    A = np.zeros((C, 3, 3))
    A[:, 0, 0] = c
    A[:, 0, 1] = s
    A[:, 1, 0] = -s
    A[:, 1, 1] = c
    A[:, 2, 0] = med[:, 0] + tr[:, 0] - c * med[:, 0] + s * med[:, 1]
    A[:, 2, 1] = med[:, 1] + tr[:, 1] - s * med[:, 0] - c * med[:, 1]
    A[:, 2, 2] = 1.0

    G = np.einsum("ikl,lm->ikm", A, W.reshape(3, C * H))
    return G.reshape(C, 3, C, H), hulT, degenerate


def _kd_split(p, ids, parts):
    """Split index array ids into `parts` groups of near-equal size (each
    <= ceil(len/parts)) by recursive median cuts on the wider dimension."""
    if parts == 1:
        return [ids]
    q = p[ids]
    dim = 0 if np.ptp(q[:, 0]) >= np.ptp(q[:, 1]) else 1
    order = ids[np.argsort(q[:, dim], kind="stable")]
    pl = parts // 2
    k = (len(order) * pl + parts - 1) // parts
    return _kd_split(p, order[:k], pl) + _kd_split(p, order[k:], parts - pl)


def _plan_and_pack(pc, ph, med, ang, tr, cm, hm):
    """Returns (cfg, in_maps): cfg=(nbank, mxbank, host_extra)."""
    med64 = med.astype(np.float64)
    ang64 = ang.astype(np.float64)
    tr64 = tr.astype(np.float64)
    G, hulT, degen = _host_coeffs(ph.astype(np.float64), med64, ang64, tr64, hm)
    hull_ok = hm.sum(-1) >= 3

    host_deep = 0.0
    boundary = []   # (i, chunk_pts_idx (np array of <=128), j)
    interior = []
    for i in range(C):
        valid = np.nonzero(cm[i])[0]
        if len(valid) == 0:
            continue
        parts = (len(valid) + P - 1) // P
        Gi = G[i].reshape(3, C * H)
        for ch in _kd_split(pc[i].astype(np.float64), valid, parts):
            q = pc[i, ch].astype(np.float64)
            qmin, qmax = q.min(0), q.max(0)
            corners = np.array(
                [[qmin[0], qmin[1], 1.0], [qmin[0], qmax[1], 1.0],
                 [qmax[0], qmin[1], 1.0], [qmax[0], qmax[1], 1.0]])
            sc = (corners @ Gi).reshape(4, C, H)
            neg_edge = (sc.max(0) < -TAU).any(-1)
            pos_edge = (sc.min(0) > TAU).any(-1)
            prunable = neg_edge & pos_edge
            env_lo_min = sc.min(-1).min(0)          # (C,) box-min of min_h s
            for j in range(C):
                if j == i or not hull_ok[j]:
                    continue
                if not degen[j] and prunable[j]:
                    continue
                if degen[j] or env_lo_min[j] >= DEEP:
                    host_deep += float(len(ch))
                    continue
                if env_lo_min[j] >= POSM:
                    interior.append((i, ch, j))
                else:
                    boundary.append((i, ch, j))

    per_b = [boundary[c::NCORES] for c in range(NCORES)]
    per_i = [interior[c::NCORES] for c in range(NCORES)]
    max_b = max(len(x) for x in per_b)
    max_s = max(len(b) + len(t) for b, t in zip(per_b, per_i))
    mxbank = (max_b + SPB - 1) // SPB
    nbank = max((max_s + SPB - 1) // SPB, mxbank)
    nbank += nbank % 2        # even # banks -> two equal groups
    assert nbank <= 6, f"PSUM budget exceeded: nbank={nbank}"

    in_maps = []
    for c in range(NCORES):
        slots = per_b[c] + per_i[c]
        lhs = np.zeros((P, nbank * P), np.float32)
        rhs = np.zeros((P, nbank * BANKW), np.float32)
        cm3 = np.zeros((P, nbank * SPB), np.float32)
        for b in range(nbank):
            tri = {}
            for si, (i, ch, j) in enumerate(slots[b * SPB:(b + 1) * SPB]):
                key = (i, ch.tobytes())
                if key not in tri:
                    t = tri[key] = len(tri)
                    n = len(ch)
                    lhs[3 * t + 0, b * P: b * P + n] = pc[i, ch, 0]
                    lhs[3 * t + 1, b * P: b * P + n] = pc[i, ch, 1]
                    lhs[3 * t + 0, b * P + n:(b + 1) * P] = SENT
                    lhs[3 * t + 1, b * P + n:(b + 1) * P] = SENT
                    lhs[3 * t + 2, b * P:(b + 1) * P] = 1.0
                t = tri[key]
                co = b * BANKW + si * H
                rhs[3 * t: 3 * t + 3, co: co + H] = G[i, :, j, :]
                cm3[: len(ch), b * SPB + si] = 1.0
        in_maps.append({
            "lhs": np.ascontiguousarray(lhs),
            "rhs": np.ascontiguousarray(rhs),
            "cmask": np.ascontiguousarray(cm3),
        })
    return (nbank, mxbank, host_deep), in_maps


def _build_nc(cfg, reps=1, loop=None):
    import concourse.bacc as bacc
    import concourse.mybir as mybir
    from concourse.tile import TileContext

    nbank, mxbank = cfg[0], cfg[1]
    f32 = mybir.dt.float32
    f32r = mybir.dt.float32r
    nc = bacc.Bacc()

    NS = nbank * SPB
    lhs_d = nc.dram_tensor("lhs", [P, nbank * P], f32r, kind="ExternalInput")
    rhs_d = nc.dram_tensor("rhs", [P, nbank * BANKW], f32r, kind="ExternalInput")
    cm_d = nc.dram_tensor("cmask", [P, NS], f32, kind="ExternalInput")
    out_d = nc.dram_tensor("out", [1, 1], f32, kind="ExternalOutput")

    GB = nbank // 2   # banks per PSUM group (2 groups, bufs=2)

    import os as _os
    unroll = int(_os.environ.get("UNROLL", str(UNROLL))) if loop is not None else 1

    with TileContext(nc) as tc:
        with tc.tile_pool(name="const", bufs=1) as cpool, \
             tc.tile_pool(name="work", bufs=2) as wpool, \
             tc.tile_pool(name="psum", bufs=2, space="PSUM") as ppool:

            sp = mybir.EngineType.SP
            lhs_sb = cpool.tile_from(lhs_d[:, :], forced_dma_engine=sp)
            rhs_sb = cpool.tile_from(rhs_d[:, :], forced_dma_engine=sp)
            cm_sb = cpool.tile_from(cm_d[:, :], forced_dma_engine=sp)
            vstrip = cpool.tile([P, NS], f32)
            ones_sb = cpool.tile([P, 1], f32)
            nc.vector.memset(ones_sb, 1.0)

            def body():
                mnstrip = wpool.tile([P, NS], f32, tag="mn")
                nmxstrip = wpool.tile([P, NS], f32, tag="nmx")
                v_t = wpool.tile([P, NS], f32, tag="v")
                w_t = wpool.tile([P, NS], f32, tag="w")
                gm_t = wpool.tile([P, NS], f32, tag="gm")
                for grp in range(2):
                    ps = ppool.tile([P, GB * BANKW], f32, tag="ps")
                    for gb in range(GB):
                        b = grp * GB + gb
                        nc.tensor.matmul(
                            ps[:, gb * BANKW: gb * BANKW + SPB * H],
                            lhs_sb[0:3 * SPB, b * P:(b + 1) * P],
                            rhs_sb[0:3 * SPB,
                                   b * BANKW: b * BANKW + SPB * H],
                            start=True, stop=True,
                        )
                    # one 4D-view reduce per op covering the whole group
                    view = ps.rearrange("p (b k) -> p b k", b=GB)[:, :, 0:SPB * H] \
                        .rearrange("p b (s h) -> p b s h", h=H)
                    so = grp * GB * SPB
                    sw = GB * SPB
                    nc.vector.tensor_reduce(
                        out=mnstrip[:, so:so + sw], in_=view,
                        axis=mybir.AxisListType.X, op=mybir.AluOpType.min,
                    )
                    mxb = min(mxbank - grp * GB, GB)   # banks needing max
                    if mxb > 0:
                        mview = view if mxb == GB else \
                            ps.rearrange("p (b k) -> p b k", b=GB)[:, 0:mxb, 0:SPB * H] \
                              .rearrange("p b (s h) -> p b s h", h=H)
                        nc.vector.tensor_reduce(
                            out=nmxstrip[:, so:so + mxb * SPB], in_=mview,
                            axis=mybir.AxisListType.X,
                            op=mybir.AluOpType.max, negate=True,
                        )
                # v = max(mn, -mx); for mx-less slots v = mn
                MXS = mxbank * SPB
                nc.vector.tensor_tensor(
                    out=v_t[:, 0:MXS], in0=mnstrip[:, 0:MXS],
                    in1=nmxstrip[:, 0:MXS], op=mybir.AluOpType.max)
                if MXS < NS:
                    nc.scalar.copy(out=v_t[:, MXS:NS], in_=mnstrip[:, MXS:NS])
                # sigmoid on ACT in parallel with the DVE gate*cmask chain
                nc.scalar.activation(
                    out=w_t, in_=v_t, func=mybir.ActivationFunctionType.Sigmoid)
                nc.vector.tensor_scalar(
                    out=gm_t, in0=v_t, scalar1=-float(EPS), scalar2=None,
                    op0=mybir.AluOpType.is_ge)
                nc.gpsimd.tensor_tensor(
                    out=gm_t, in0=gm_t, in1=cm_sb, op=mybir.AluOpType.mult)
                nc.gpsimd.tensor_tensor(
                    out=vstrip, in0=w_t, in1=gm_t, op=mybir.AluOpType.mult)

            if loop is not None:
                stg = _os.environ.get("LOOP_STAGGERED", "0") == "1"
                with tc.For_i(0, loop, 1, staggered_reset=stg) as _i:
                    for _ in range(unroll):
                        body()
            else:
                for _ in range(reps):
                    body()

            acc = cpool.tile([P, 1], f32)
            nc.vector.tensor_reduce(
                out=acc, in_=vstrip, axis=mybir.AxisListType.X,
                op=mybir.AluOpType.add,
            )
            out_ps = ppool.tile([1, 1], f32, tag="ps2")
            nc.tensor.matmul(out_ps, acc, ones_sb, start=True, stop=True)
            out_sb = cpool.tile([1, 1], f32)
            nc.scalar.copy(out=out_sb, in_=out_ps)
            nc.sync.dma_start(out=out_d[:, :], in_=out_sb)

    nc.compile()
    return nc


def kernel(padded_clusters, padded_hulls, medoids, rotation_angles,
           translations, cluster_masks, hull_masks):
    pc = np.asarray(padded_clusters, dtype=np.float32)
    ph = np.asarray(padded_hulls, dtype=np.float32)
    med = np.asarray(medoids, dtype=np.float32)
    ang = np.asarray(rotation_angles, dtype=np.float32)
    tr = np.asarray(translations, dtype=np.float32)
    cm = np.asarray(cluster_masks)
    hm = np.asarray(hull_masks)

    cfg, in_maps = _plan_and_pack(pc, ph, med, ang, tr, cm, hm)

    key = ("nc", cfg[0], cfg[1])
    if key not in _NC_CACHE:
        _NC_CACHE[key] = _build_nc(cfg)
    nc = _NC_CACHE[key]

    from concourse.bass_utils import run_bass_kernel_spmd
    res = run_bass_kernel_spmd(nc, in_maps, core_ids=list(range(NCORES)))
    _NC_CACHE["last_results"] = res

    sep = sum(float(r["out"][0, 0]) for r in res.results) + cfg[2]
    total = (SEP_W * sep
             + T_PEN * float(np.sum(tr.astype(np.float64) ** 2))
             + R_PEN * float(np.sum(ang.astype(np.float64) ** 2)))
    return np.asarray(total, dtype=np.float32)


# revision 11
# speedup vs baseline: 2.7255x; 1.0737x over previous
"""Trainium2 Bass kernel for ClusterSeparationOptimizer (v2).

Math (identical to reference up to fp32 rounding):
  signed[i,n,j,h] = [x, y, 1] @ (A_i @ W[:, j, h])   (affine in the RAW point)
  mn = min_h signed, mx = max_h signed               (over valid edges)
  v  = max(mn, -mx)   -> v >= -EPS iff inside; v = min|signed| when inside
  viol = sigmoid(v) * (v >= -EPS) * cluster_mask
  out  = sum viol (i!=j, hull_ok) + 0.1*|translations|^2 + |angles|^2

Host-side planning (fp64, exact):
  * Only VALID points are packed: each cluster's n_i real points are
    kd-split into ceil(n_i/128) chunks of <=128; chunks padded to 128 with
    far sentinels (cmask=0, v<0 there by convexity).
  * Hull orientation is normalized (W flipped so interior => all s > 0).
  * Per (chunk, hull) pair, exact corner tests on the chunk bbox (signed is
    affine in the raw point; env_lo=min_h s is concave so its box-min is at
    a corner):
      - pruned   : some edge all-corners < -TAU and some all > TAU
                   -> every point sign-mixed -> viol == 0.
      - deep     : env_lo >= DEEP at all corners -> sigmoid(mn) = 1 within
                   e^-DEEP per point; host adds count*1.0, pair skipped.
      - interior : env_lo >= POSM at all corners -> mx > 0 > -mx <= mn, so
                   v = mn exactly; the device skips the max-reduce.
      - boundary : both reduces.

Device (SPMD one program, per-core data):
  Pairs are packed as 40-wide column slots, 12 per PSUM bank.  One
  float32r matmul per bank: lhsT[K<=36,128] holds [x,y,1] of each slot's
  chunk K-triple (block-diagonal rhs holds each slot's 40 G columns), so a
  single wide (480-col, >=256 => 1 cycle/row) matmul computes 12 slots'
  signed distances for 128 points.  Banks are processed in 2 groups of 3
  with a bufs=2 PSUM pool so group g+1 matmuls overlap group g reduces.
  Per bank: DVE tensor_reduce(min) -> mn strip; for the first mxbank banks
  (boundary slots first) tensor_reduce(max, negate) -> -mx strip; the
  interior tail of the -mx strip is pre-set to -BIG once.  Tail:
  v = max(mn, nmx); sigmoid on ACT; (v >= -EPS) gate; * cmask -> vstrip.
  Final: reduce_sum + ones-matmul -> scalar; host all-reduces the 8 cores
  and adds the deep-interior count and penalty terms.
"""

import numpy as np

C, N, H = 24, 1536, 40
NCORES = 8
P = 128                    # points per chunk / partition dim
SPB = 12                   # slots per 512-col PSUM bank (12*40=480)
BANKW = 512
SEP_W, T_PEN, R_PEN = 1.0, 0.1, 1.0
EPS = 1e-8
BIG = 1e30
TAU = 1e-5                 # prune margin
POSM = 1e-2                # interior margin (device fp32 slop ~1e-4)
DEEP = 8.5                 # deep-interior skip: per-point err <= e^-8.5
SENT = 1.0e6               # sentinel coordinate for padded points
UNROLL = 8                 # bodies per For_i iteration (timing loop only)

_NC_CACHE = {}


def _transform64(x, med, ang, tr):
    c, s = np.cos(ang), np.sin(ang)
    xc = x[..., 0] - med[:, None, 0]
    yc = x[..., 1] - med[:, None, 1]
    px = c[:, None] * xc - s[:, None] * yc + (med[:, 0] + tr[:, 0])[:, None]
    py = s[:, None] * xc + c[:, None] * yc + (med[:, 1] + tr[:, 1])[:, None]
    return np.stack([px, py], -1)


def _host_coeffs(ph, med, ang, tr, hm):
    """G[i] = A_i @ W: (C, 3, C, H) float64; rows act on raw [x, y, 1].

    W is orientation-normalized so that hull interiors have s > 0."""
    hulT = _transform64(ph, med, ang, tr)
    hx, hy = hulT[..., 0], hulT[..., 1]
    ex = np.roll(hx, -1, axis=1) - hx
    ey = np.roll(hy, -1, axis=1) - hy
    elen_raw = np.sqrt(ex * ex + ey * ey)
    elen = elen_raw + EPS
    evalid = elen_raw > 1e-6
    a = ex / elen
    b = -ey / elen
    d = -(ex * hy - ey * hx) / elen

    W = np.stack([b, a, d], axis=0)  # (3, C, H): coeffs on transformed [x,y,1]
    degenerate = np.zeros(C, bool)
    flip = np.ones(C)
    for j in range(C):
        inv = ~evalid[j]
        val = np.nonzero(evalid[j])[0]
        if inv.any():
            if len(val) > 0:
                W[:, j, inv] = W[:, j, val[-1]][:, None]
            else:
                W[:, j, :] = np.array([0.0, 0.0, BIG])[:, None]
                degenerate[j] = True
        if not degenerate[j]:
            vm = hm[j] if hm[j].any() else np.ones(H, bool)
            cx, cy = hulT[j, vm, 0].mean(), hulT[j, vm, 1].mean()
            sc = W[0, j, val] * cx + W[1, j, val] * cy + W[2, j, val]
            if np.median(sc) < 0:
                flip[j] = -1.0
                W[:, j, :] = -W[:, j, :]

    c, s = np.cos(ang), np.sin(ang)
    A = np.zeros((C, 3, 3))
    A[:, 0, 0] = c
    A[:, 0, 1] = s
    A[:, 1, 0] = -s
    A[:, 1, 1] = c
    A[:, 2, 0] = med[:, 0] + tr[:, 0] - c * med[:, 0] + s * med[:, 1]
    A[:, 2, 1] = med[:, 1] + tr[:, 1] - s * med[:, 0] - c * med[:, 1]
    A[:, 2, 2] = 1.0

    G = np.einsum("ikl,lm->ikm", A, W.reshape(3, C * H))
    return G.reshape(C, 3, C, H), hulT, degenerate


def _kd_split(p, ids, parts):
    """Split index array ids into `parts` groups of near-equal size (each
    <= ceil(len/parts)) by recursive median cuts on the wider dimension."""
    if parts == 1:
        return [ids]
    q = p[ids]
    dim = 0 if np.ptp(q[:, 0]) >= np.ptp(q[:, 1]) else 1
    order = ids[np.argsort(q[:, dim], kind="stable")]
    pl = parts // 2
    k = (len(order) * pl + parts - 1) // parts
    return _kd_split(p, order[:k], pl) + _kd_split(p, order[k:], parts - pl)


def _plan_and_pack(pc, ph, med, ang, tr, cm, hm):
    """Returns (cfg, in_maps): cfg=(nbank, mxbank, host_extra)."""
    med64 = med.astype(np.float64)
    ang64 = ang.astype(np.float64)
    tr64 = tr.astype(np.float64)
    G, hulT, degen = _host_coeffs(ph.astype(np.float64), med64, ang64, tr64, hm)
    hull_ok = hm.sum(-1) >= 3

    host_deep = 0.0
    boundary = []   # (i, chunk_pts_idx (np array of <=128), j)
    interior = []
    for i in range(C):
        valid = np.nonzero(cm[i])[0]
        if len(valid) == 0:
            continue
        parts = (len(valid) + P - 1) // P
        Gi = G[i].reshape(3, C * H)
        for ch in _kd_split(pc[i].astype(np.float64), valid, parts):
            q = pc[i, ch].astype(np.float64)
            qmin, qmax = q.min(0), q.max(0)
            corners = np.array(
                [[qmin[0], qmin[1], 1.0], [qmin[0], qmax[1], 1.0],
                 [qmax[0], qmin[1], 1.0], [qmax[0], qmax[1], 1.0]])
            sc = (corners @ Gi).reshape(4, C, H)
            neg_edge = (sc.max(0) < -TAU).any(-1)
            pos_edge = (sc.min(0) > TAU).any(-1)
            prunable = neg_edge & pos_edge
            env_lo_min = sc.min(-1).min(0)          # (C,) box-min of min_h s
            for j in range(C):
                if j == i or not hull_ok[j]:
                    continue
                if not degen[j] and prunable[j]:
                    continue
                if degen[j] or env_lo_min[j] >= DEEP:
                    host_deep += float(len(ch))
                    continue
                if env_lo_min[j] >= POSM:
                    interior.append((i, ch, j))
                else:
                    boundary.append((i, ch, j))

    per_b = [boundary[c::NCORES] for c in range(NCORES)]
    per_i = [interior[c::NCORES] for c in range(NCORES)]
    max_b = max(len(x) for x in per_b)
    max_s = max(len(b) + len(t) for b, t in zip(per_b, per_i))
    mxbank = (max_b + SPB - 1) // SPB
    nbank = max((max_s + SPB - 1) // SPB, mxbank)
    nbank += nbank % 2        # even # banks -> two equal groups
    assert nbank <= 6, f"PSUM budget exceeded: nbank={nbank}"

    in_maps = []
    for c in range(NCORES):
        slots = per_b[c] + per_i[c]
        lhs = np.zeros((P, nbank * P), np.float32)
        rhs = np.zeros((P, nbank * BANKW), np.float32)
        cm3 = np.zeros((P, nbank * SPB), np.float32)
        for b in range(nbank):
            tri = {}
            for si, (i, ch, j) in enumerate(slots[b * SPB:(b + 1) * SPB]):
                key = (i, ch.tobytes())
                if key not in tri:
                    t = tri[key] = len(tri)
                    n = len(ch)
                    lhs[3 * t + 0, b * P: b * P + n] = pc[i, ch, 0]
                    lhs[3 * t + 1, b * P: b * P + n] = pc[i, ch, 1]
                    lhs[3 * t + 0, b * P + n:(b + 1) * P] = SENT
                    lhs[3 * t + 1, b * P + n:(b + 1) * P] = SENT
                    lhs[3 * t + 2, b * P:(b + 1) * P] = 1.0
                t = tri[key]
                co = b * BANKW + si * H
                rhs[3 * t: 3 * t + 3, co: co + H] = G[i, :, j, :]
                cm3[: len(ch), b * SPB + si] = 1.0
        in_maps.append({
            "lhs": np.ascontiguousarray(lhs),
            "rhs": np.ascontiguousarray(rhs),
            "cmask": np.ascontiguousarray(cm3),
        })
    return (nbank, mxbank, host_deep), in_maps


def _build_nc(cfg, reps=1, loop=None):
    import concourse.bacc as bacc
    import concourse.mybir as mybir
    from concourse.tile import TileContext

    nbank, mxbank = cfg[0], cfg[1]
    f32 = mybir.dt.float32
    f32r = mybir.dt.float32r
    nc = bacc.Bacc()

    NS = nbank * SPB
    lhs_d = nc.dram_tensor("lhs", [P, nbank * P], f32r, kind="ExternalInput")
    rhs_d = nc.dram_tensor("rhs", [P, nbank * BANKW], f32r, kind="ExternalInput")
    cm_d = nc.dram_tensor("cmask", [P, NS], f32, kind="ExternalInput")
    out_d = nc.dram_tensor("out", [1, 1], f32, kind="ExternalOutput")

    GB = nbank // 2   # banks per PSUM group (2 groups, bufs=2)

    import os as _os
    unroll = int(_os.environ.get("UNROLL", str(UNROLL))) if loop is not None else 1

    with TileContext(nc) as tc:
        with tc.tile_pool(name="const", bufs=1) as cpool, \
             tc.tile_pool(name="work", bufs=2) as wpool, \
             tc.tile_pool(name="psum", bufs=2, space="PSUM") as ppool:

            sp = mybir.EngineType.SP
            lhs_sb = cpool.tile_from(lhs_d[:, :], forced_dma_engine=sp)
            rhs_sb = cpool.tile_from(rhs_d[:, :], forced_dma_engine=sp)
            cm_sb = cpool.tile_from(cm_d[:, :], forced_dma_engine=sp)
            vstrip = cpool.tile([P, NS], f32)
            ones_sb = cpool.tile([P, 1], f32)
            nc.vector.memset(ones_sb, 1.0)

            def body():
                mnstrip = wpool.tile([P, NS], f32, tag="mn")
                nmxstrip = wpool.tile([P, NS], f32, tag="nmx")
                v_t = wpool.tile([P, NS], f32, tag="v")
                w_t = wpool.tile([P, NS], f32, tag="w")
                gm_t = wpool.tile([P, NS], f32, tag="gm")
                for grp in range(2):
                    ps = ppool.tile([P, GB * BANKW], f32, tag="ps")
                    for gb in range(GB):
                        b = grp * GB + gb
                        nc.tensor.matmul(
                            ps[:, gb * BANKW: gb * BANKW + SPB * H],
                            lhs_sb[0:3 * SPB, b * P:(b + 1) * P],
                            rhs_sb[0:3 * SPB,
                                   b * BANKW: b * BANKW + SPB * H],
                            start=True, stop=True,
                        )
                    # one 4D-view reduce per op covering the whole group
                    view = ps.rearrange("p (b k) -> p b k", b=GB)[:, :, 0:SPB * H] \
                        .rearrange("p b (s h) -> p b s h", h=H)
                    so = grp * GB * SPB
                    sw = GB * SPB
                    nc.vector.tensor_reduce(
                        out=mnstrip[:, so:so + sw], in_=view,
                        axis=mybir.AxisListType.X, op=mybir.AluOpType.min,
                    )
                    mxb = min(mxbank - grp * GB, GB)   # banks needing max
                    if mxb > 0:
                        mview = view if mxb == GB else \
                            ps.rearrange("p (b k) -> p b k", b=GB)[:, 0:mxb, 0:SPB * H] \
                              .rearrange("p b (s h) -> p b s h", h=H)
                        nc.vector.tensor_reduce(
                            out=nmxstrip[:, so:so + mxb * SPB], in_=mview,
                            axis=mybir.AxisListType.X,
                            op=mybir.AluOpType.max, negate=True,
                        )
                # v = max(mn, -mx); for mx-less slots v = mn
                MXS = mxbank * SPB
                nc.vector.tensor_tensor(
                    out=v_t[:, 0:MXS], in0=mnstrip[:, 0:MXS],
                    in1=nmxstrip[:, 0:MXS], op=mybir.AluOpType.max)
                if MXS < NS:
                    nc.scalar.copy(out=v_t[:, MXS:NS], in_=mnstrip[:, MXS:NS])
                # sigmoid on ACT in parallel with the DVE gate*cmask chain
                nc.scalar.activation(
                    out=w_t, in_=v_t, func=mybir.ActivationFunctionType.Sigmoid)
                nc.vector.tensor_scalar(
                    out=gm_t, in0=v_t, scalar1=-float(EPS), scalar2=None,
                    op0=mybir.AluOpType.is_ge)
                nc.gpsimd.tensor_tensor(
                    out=gm_t, in0=gm_t, in1=cm_sb, op=mybir.AluOpType.mult)
                nc.gpsimd.tensor_tensor(
                    out=vstrip, in0=w_t, in1=gm_t, op=mybir.AluOpType.mult)

            if loop is not None:
                stg = _os.environ.get("LOOP_STAGGERED", "0") == "1"
                with tc.For_i(0, loop, 1, staggered_reset=stg) as _i:
                    for _ in range(unroll):
                        body()
            else:
                for _ in range(reps):
                    body()

            acc = cpool.tile([P, 1], f32)
            nc.vector.tensor_reduce(
                out=acc, in_=vstrip, axis=mybir.AxisListType.X,
                op=mybir.AluOpType.add,
            )
            out_ps = ppool.tile([1, 1], f32, tag="ps2")
            nc.tensor.matmul(out_ps, acc, ones_sb, start=True, stop=True)
            out_sb = cpool.tile([1, 1], f32)
            nc.scalar.copy(out=out_sb, in_=out_ps)
            nc.sync.dma_start(out=out_d[:, :], in_=out_sb)

    nc.compile()
    return nc


def kernel(padded_clusters, padded_hulls, medoids, rotation_angles,
           translations, cluster_masks, hull_masks):
    pc = np.asarray(padded_clusters, dtype=np.float32)
    ph = np.asarray(padded_hulls, dtype=np.float32)
    med = np.asarray(medoids, dtype=np.float32)
    ang = np.asarray(rotation_angles, dtype=np.float32)
    tr = np.asarray(translations, dtype=np.float32)
    cm = np.asarray(cluster_masks)
    hm = np.asarray(hull_masks)

    cfg, in_maps = _plan_and_pack(pc, ph, med, ang, tr, cm, hm)

    key = ("nc", cfg[0], cfg[1])
    if key not in _NC_CACHE:
        _NC_CACHE[key] = _build_nc(cfg)
    nc = _NC_CACHE[key]

    from concourse.bass_utils import run_bass_kernel_spmd
    res = run_bass_kernel_spmd(nc, in_maps, core_ids=list(range(NCORES)))
    _NC_CACHE["last_results"] = res

    sep = sum(float(r["out"][0, 0]) for r in res.results) + cfg[2]
    total = (SEP_W * sep
             + T_PEN * float(np.sum(tr.astype(np.float64) ** 2))
             + R_PEN * float(np.sum(ang.astype(np.float64) ** 2)))
    return np.asarray(total, dtype=np.float32)


# revision 14
# speedup vs baseline: 4.8478x; 1.7787x over previous
"""Trainium2 Bass kernel for ClusterSeparationOptimizer (v2).

Math (identical to reference up to fp32 rounding):
  signed[i,n,j,h] = [x, y, 1] @ (A_i @ W[:, j, h])   (affine in the RAW point)
  mn = min_h signed, mx = max_h signed               (over valid edges)
  v  = max(mn, -mx)   -> v >= -EPS iff inside; v = min|signed| when inside
  viol = sigmoid(v) * (v >= -EPS) * cluster_mask
  out  = sum viol (i!=j, hull_ok) + 0.1*|translations|^2 + |angles|^2

Host-side planning (fp64, exact):
  * Only VALID points are packed: each cluster's n_i real points are
    kd-split into ceil(n_i/128) chunks of <=128; chunks padded to 128 with
    far sentinels (cmask=0, v<0 there by convexity).
  * Hull orientation is normalized (W flipped so interior => all s > 0).
  * Per (chunk, hull) pair, exact corner tests on the chunk bbox (signed is
    affine in the raw point; env_lo=min_h s is concave so its box-min is at
    a corner):
      - pruned   : some edge all-corners < -TAU and some all > TAU
                   -> every point sign-mixed -> viol == 0.
      - deep     : env_lo >= DEEP at all corners -> sigmoid(mn) = 1 within
                   e^-DEEP per point; host adds count*1.0, pair skipped.
      - interior : env_lo >= POSM at all corners -> mx > 0 > -mx <= mn, so
                   v = mn exactly; the device skips the max-reduce.
      - boundary : both reduces.

Device (SPMD one program, per-core data):
  Pairs are packed as 40-wide column slots, 12 per PSUM bank.  One
  float32r matmul per bank: lhsT[K<=36,128] holds [x,y,1] of each slot's
  chunk K-triple (block-diagonal rhs holds each slot's 40 G columns), so a
  single wide (480-col, >=256 => 1 cycle/row) matmul computes 12 slots'
  signed distances for 128 points.  Banks are processed in 2 groups of 3
  with a bufs=2 PSUM pool so group g+1 matmuls overlap group g reduces.
  Per bank: DVE tensor_reduce(min) -> mn strip; for the first mxbank banks
  (boundary slots first) tensor_reduce(max, negate) -> -mx strip; the
  interior tail of the -mx strip is pre-set to -BIG once.  Tail:
  v = max(mn, nmx); sigmoid on ACT; (v >= -EPS) gate; * cmask -> vstrip.
  Final: reduce_sum + ones-matmul -> scalar; host all-reduces the 8 cores
  and adds the deep-interior count and penalty terms.
"""

import numpy as np

C, N, H = 24, 1536, 40
NCORES = 8
P = 128                    # points per chunk / partition dim
SPB = 12                   # slots per 512-col PSUM bank (12*40=480)
BANKW = 512
SEP_W, T_PEN, R_PEN = 1.0, 0.1, 1.0
EPS = 1e-8
BIG = 1e30
TAU = 1e-5                 # prune margin
POSM = 1e-2                # interior margin (device fp32 slop ~1e-4)
DEEP = 8.5                 # deep-interior skip: per-point err <= e^-8.5
SENT = 1.0e6               # sentinel coordinate for padded points
UNROLL = 8                 # bodies per For_i iteration (timing loop only)

_NC_CACHE = {}


def _transform64(x, med, ang, tr):
    c, s = np.cos(ang), np.sin(ang)
    xc = x[..., 0] - med[:, None, 0]
    yc = x[..., 1] - med[:, None, 1]
    px = c[:, None] * xc - s[:, None] * yc + (med[:, 0] + tr[:, 0])[:, None]
    py = s[:, None] * xc + c[:, None] * yc + (med[:, 1] + tr[:, 1])[:, None]
    return np.stack([px, py], -1)


def _host_coeffs(ph, med, ang, tr, hm):
    """G[i] = A_i @ W: (C, 3, C, H) float64; rows act on raw [x, y, 1].

    W is orientation-normalized so that hull interiors have s > 0."""
    hulT = _transform64(ph, med, ang, tr)
    hx, hy = hulT[..., 0], hulT[..., 1]
    ex = np.roll(hx, -1, axis=1) - hx
    ey = np.roll(hy, -1, axis=1) - hy
    elen_raw = np.sqrt(ex * ex + ey * ey)
    elen = elen_raw + EPS
    evalid = elen_raw > 1e-6
    a = ex / elen
    b = -ey / elen
    d = -(ex * hy - ey * hx) / elen

    W = np.stack([b, a, d], axis=0)  # (3, C, H): coeffs on transformed [x,y,1]
    degenerate = np.zeros(C, bool)
    flip = np.ones(C)
    for j in range(C):
        inv = ~evalid[j]
        val = np.nonzero(evalid[j])[0]
        if inv.any():
            if len(val) > 0:
                W[:, j, inv] = W[:, j, val[-1]][:, None]
            else:
                W[:, j, :] = np.array([0.0, 0.0, BIG])[:, None]
                degenerate[j] = True
        if not degenerate[j]:
            vm = hm[j] if hm[j].any() else np.ones(H, bool)
            cx, cy = hulT[j, vm, 0].mean(), hulT[j, vm, 1].mean()
            sc = W[0, j, val] * cx + W[1, j, val] * cy + W[2, j, val]
            if np.median(sc) < 0:
                flip[j] = -1.0
                W[:, j, :] = -W[:, j, :]

    c, s = np.cos(ang), np.sin(ang)
    A = np.zeros((C, 3, 3))
    A[:, 0, 0] = c
    A[:, 0, 1] = s
    A[:, 1, 0] = -s
    A[:, 1, 1] = c
    A[:, 2, 0] = med[:, 0] + tr[:, 0] - c * med[:, 0] + s * med[:, 1]
    A[:, 2, 1] = med[:, 1] + tr[:, 1] - s * med[:, 0] - c * med[:, 1]
    A[:, 2, 2] = 1.0

    G = np.einsum("ikl,lm->ikm", A, W.reshape(3, C * H))
    return G.reshape(C, 3, C, H), hulT, degenerate


def _kd_split(p, ids, parts):
    """Split index array ids into `parts` groups of near-equal size (each
    <= ceil(len/parts)) by recursive median cuts on the wider dimension."""
    if parts == 1:
        return [ids]
    q = p[ids]
    dim = 0 if np.ptp(q[:, 0]) >= np.ptp(q[:, 1]) else 1
    order = ids[np.argsort(q[:, dim], kind="stable")]
    pl = parts // 2
    k = (len(order) * pl + parts - 1) // parts
    return _kd_split(p, order[:k], pl) + _kd_split(p, order[k:], parts - pl)


def _plan_and_pack(pc, ph, med, ang, tr, cm, hm):
    """Returns (cfg, in_maps): cfg=(nbank, mxbank, host_extra)."""
    med64 = med.astype(np.float64)
    ang64 = ang.astype(np.float64)
    tr64 = tr.astype(np.float64)
    G, hulT, degen = _host_coeffs(ph.astype(np.float64), med64, ang64, tr64, hm)
    hull_ok = hm.sum(-1) >= 3

    host_deep = 0.0
    boundary = []   # (i, chunk_pts_idx (np array of <=128), j)
    interior = []
    for i in range(C):
        valid = np.nonzero(cm[i])[0]
        if len(valid) == 0:
            continue
        parts = (len(valid) + P - 1) // P
        Gi = G[i].reshape(3, C * H)
        for ch in _kd_split(pc[i].astype(np.float64), valid, parts):
            q = pc[i, ch].astype(np.float64)
            qmin, qmax = q.min(0), q.max(0)
            corners = np.array(
                [[qmin[0], qmin[1], 1.0], [qmin[0], qmax[1], 1.0],
                 [qmax[0], qmin[1], 1.0], [qmax[0], qmax[1], 1.0]])
            sc = (corners @ Gi).reshape(4, C, H)
            neg_edge = (sc.max(0) < -TAU).any(-1)
            pos_edge = (sc.min(0) > TAU).any(-1)
            prunable = neg_edge & pos_edge
            env_lo_min = sc.min(-1).min(0)          # (C,) box-min of min_h s
            for j in range(C):
                if j == i or not hull_ok[j]:
                    continue
                if not degen[j] and prunable[j]:
                    continue
                if degen[j] or env_lo_min[j] >= DEEP:
                    host_deep += float(len(ch))
                    continue
                if env_lo_min[j] >= POSM:
                    interior.append((i, ch, j))
                else:
                    boundary.append((i, ch, j))

    pairs = boundary + interior
    per_core = [pairs[c::NCORES] for c in range(NCORES)]
    max_s = max(len(x) for x in per_core)
    mxbank = 0                # no max pass needed (see _build_nc)
    nbank = (max_s + SPB - 1) // SPB
    nbank += nbank % 2        # even # banks -> two equal groups
    assert nbank <= 6, f"PSUM budget exceeded: nbank={nbank}"

    in_maps = []
    for c in range(NCORES):
        slots = per_core[c]
        lhs = np.zeros((P, nbank * P), np.float32)
        rhs = np.zeros((P, nbank * BANKW), np.float32)
        cm3 = np.zeros((P, nbank * SPB), np.float32)
        for b in range(nbank):
            tri = {}
            for si, (i, ch, j) in enumerate(slots[b * SPB:(b + 1) * SPB]):
                key = (i, ch.tobytes())
                if key not in tri:
                    t = tri[key] = len(tri)
                    n = len(ch)
                    lhs[3 * t + 0, b * P: b * P + n] = pc[i, ch, 0]
                    lhs[3 * t + 1, b * P: b * P + n] = pc[i, ch, 1]
                    lhs[3 * t + 0, b * P + n:(b + 1) * P] = SENT
                    lhs[3 * t + 1, b * P + n:(b + 1) * P] = SENT
                    lhs[3 * t + 2, b * P:(b + 1) * P] = 1.0
                t = tri[key]
                co = b * BANKW + si * H
                rhs[3 * t: 3 * t + 3, co: co + H] = G[i, :, j, :]
                cm3[: len(ch), b * SPB + si] = 1.0
        in_maps.append({
            "lhs": np.ascontiguousarray(lhs),
            "rhs": np.ascontiguousarray(rhs),
            "cmask": np.ascontiguousarray(cm3),
        })
    return (nbank, mxbank, host_deep), in_maps


def _build_nc(cfg, reps=1, loop=None):
    import concourse.bacc as bacc
    import concourse.mybir as mybir
    from concourse.tile import TileContext

    nbank = cfg[0]
    f32 = mybir.dt.float32
    f32r = mybir.dt.float32r
    nc = bacc.Bacc()

    NS = nbank * SPB
    lhs_d = nc.dram_tensor("lhs", [P, nbank * P], f32r, kind="ExternalInput")
    rhs_d = nc.dram_tensor("rhs", [P, nbank * BANKW], f32r, kind="ExternalInput")
    cm_d = nc.dram_tensor("cmask", [P, NS], f32, kind="ExternalInput")
    out_d = nc.dram_tensor("out", [1, 1], f32, kind="ExternalOutput")

    GB = nbank // 2   # banks per PSUM group (2 groups, bufs=2)

    import os as _os
    unroll = int(_os.environ.get("UNROLL", str(UNROLL))) if loop is not None else 1

    with TileContext(nc) as tc:
        with tc.tile_pool(name="const", bufs=1) as cpool, \
             tc.tile_pool(name="work", bufs=2) as wpool, \
             tc.tile_pool(name="psum", bufs=2, space="PSUM") as ppool:

            sp = mybir.EngineType.SP
            lhs_sb = cpool.tile_from(lhs_d[:, :], forced_dma_engine=sp)
            rhs_sb = cpool.tile_from(rhs_d[:, :], forced_dma_engine=sp)
            cm_sb = cpool.tile_from(cm_d[:, :], forced_dma_engine=sp)
            vstrip = cpool.tile([P, NS], f32)
            ones_sb = cpool.tile([P, 1], f32)
            nc.vector.memset(ones_sb, 1.0)

            def body():
                mnstrip = wpool.tile([P, NS], f32, tag="mn")
                w_t = wpool.tile([P, NS], f32, tag="w")
                gm_t = wpool.tile([P, NS], f32, tag="gm")
                for grp in range(2):
                    ps = ppool.tile([P, GB * BANKW], f32, tag="ps")
                    for gb in range(GB):
                        b = grp * GB + gb
                        nc.tensor.matmul(
                            ps[:, gb * BANKW: gb * BANKW + SPB * H],
                            lhs_sb[0:3 * SPB, b * P:(b + 1) * P],
                            rhs_sb[0:3 * SPB,
                                   b * BANKW: b * BANKW + SPB * H],
                            start=True, stop=True,
                        )
                    # one 4D-view min-reduce covering the whole group.
                    # (no max pass: with inward-normalized hulls, all_neg can
                    # never fire, so inside <=> mn >= -EPS and min|s| = mn)
                    view = ps.rearrange("p (b k) -> p b k", b=GB)[:, :, 0:SPB * H] \
                        .rearrange("p b (s h) -> p b s h", h=H)
                    so = grp * GB * SPB
                    sw = GB * SPB
                    nc.vector.tensor_reduce(
                        out=mnstrip[:, so:so + sw], in_=view,
                        axis=mybir.AxisListType.X, op=mybir.AluOpType.min,
                    )
                # sigmoid on ACT in parallel with the DVE gate; muls on Pool
                nc.scalar.activation(
                    out=w_t, in_=mnstrip, func=mybir.ActivationFunctionType.Sigmoid)
                nc.vector.tensor_scalar(
                    out=gm_t, in0=mnstrip, scalar1=-float(EPS), scalar2=None,
                    op0=mybir.AluOpType.is_ge)
                nc.gpsimd.tensor_tensor(
                    out=gm_t, in0=gm_t, in1=cm_sb, op=mybir.AluOpType.mult)
                nc.gpsimd.tensor_tensor(
                    out=vstrip, in0=w_t, in1=gm_t, op=mybir.AluOpType.mult)

            if loop is not None:
                stg = _os.environ.get("LOOP_STAGGERED", "0") == "1"
                with tc.For_i(0, loop, 1, staggered_reset=stg) as _i:
                    for _ in range(unroll):
                        body()
            else:
                for _ in range(reps):
                    body()

            acc = cpool.tile([P, 1], f32)
            nc.vector.tensor_reduce(
                out=acc, in_=vstrip, axis=mybir.AxisListType.X,
                op=mybir.AluOpType.add,
            )
            out_ps = ppool.tile([1, 1], f32, tag="ps2")
            nc.tensor.matmul(out_ps, acc, ones_sb, start=True, stop=True)
            out_sb = cpool.tile([1, 1], f32)
            nc.scalar.copy(out=out_sb, in_=out_ps)
            nc.sync.dma_start(out=out_d[:, :], in_=out_sb)

    nc.compile()
    return nc


def kernel(padded_clusters, padded_hulls, medoids, rotation_angles,
           translations, cluster_masks, hull_masks):
    pc = np.asarray(padded_clusters, dtype=np.float32)
    ph = np.asarray(padded_hulls, dtype=np.float32)
    med = np.asarray(medoids, dtype=np.float32)
    ang = np.asarray(rotation_angles, dtype=np.float32)
    tr = np.asarray(translations, dtype=np.float32)
    cm = np.asarray(cluster_masks)
    hm = np.asarray(hull_masks)

    cfg, in_maps = _plan_and_pack(pc, ph, med, ang, tr, cm, hm)

    key = ("nc", cfg[0], cfg[1])
    if key not in _NC_CACHE:
        _NC_CACHE[key] = _build_nc(cfg)
    nc = _NC_CACHE[key]

    from concourse.bass_utils import run_bass_kernel_spmd
    res = run_bass_kernel_spmd(nc, in_maps, core_ids=list(range(NCORES)))
    _NC_CACHE["last_results"] = res

    sep = sum(float(r["out"][0, 0]) for r in res.results) + cfg[2]
    total = (SEP_W * sep
             + T_PEN * float(np.sum(tr.astype(np.float64) ** 2))
             + R_PEN * float(np.sum(ang.astype(np.float64) ** 2)))
    return np.asarray(total, dtype=np.float32)


# revision 16
# speedup vs baseline: 6.1090x; 1.2602x over previous
"""Trainium2 Bass kernel for ClusterSeparationOptimizer (v3).

Math (identical to reference up to fp32 rounding):
  signed[i,n,j,h] = [x, y, 1] @ (A_i @ W[:, j, h])   (affine in the RAW point)
  mn = min_h signed (over valid edges, hull orientation normalized inward)
  viol = sigmoid(mn) * (mn >= -EPS) * cluster_mask
  out  = sum viol (i!=j, hull_ok) + 0.1*|translations|^2 + |angles|^2

Why no max pass: the reference tests all_pos OR all_neg.  After host-side
orientation normalization (W flipped so hull interiors have s > 0), all_neg
can never fire: for a bounded convex polygon with inward normals, every
point of the plane lies strictly on the interior side of some (far) edge,
so max_h s_h(p) >= O(inradius) >> EPS for all p.  Hence
inside <=> mn >= -EPS, and min|s| = mn when inside (up to < EPS).

Host-side planning (fp64, exact):
  * Only VALID points are packed: each cluster's n_i real points are
    kd-split into ceil(n_i/128) chunks of <=128, padded with far sentinels
    (cmask=0 there; mn(sentinel) << 0 by convexity so they are gated off).
  * Per (chunk, hull) pair, exact corner tests on the chunk bbox (signed is
    affine in the raw point; env_lo=min_h s is concave so its box-min is at
    a corner):
      - pruned : some edge all-corners < -TAU and some all > TAU
                 -> every point sign-mixed -> viol == 0.
      - deep   : env_lo >= DEEP at all corners -> sigmoid(mn) = 1 within
                 e^-DEEP per point; host adds count*1.0, pair skipped.
  * Each surviving pair becomes 1 sub-slot (h <= 20) or 2 sub-slots of 20
    G-columns (padding edge columns repeat a valid edge, so min over a
    20-superset of the valid columns is exact).

Device (SPMD one program, per-core data):
  Sub-slots are 20-wide column groups, 25 per PSUM bank (500 cols).  One
  float32r matmul per bank (>=256 cols => 1 PE cycle/row): lhsT[K<=75,128]
  holds [x,y,1] K-triples of each sub-slot's chunk, the block-diagonal rhs
  holds the G columns.  Banks are split into 2 PSUM groups (bufs=2) so one
  group's matmuls overlap the other group's reduce.  Per group one DVE
  tensor_reduce(min) (4D view, exact sub-slot count) -> mn sub-strip.
  Tail off-DVE: Pool combines 2-sub pairs (min) into the pair strip, ACT
  copies the 1-sub region and applies sigmoid, Pool computes the
  (mn >= -EPS) gate, multiplies by cmask and by sigmoid -> vstrip.
  Final (after the timing loop): reduce_sum + ones-matmul -> scalar; the
  host all-reduces the 8 cores and adds deep-count and penalty terms.
"""

import numpy as np

C, N, H = 24, 1536, 40
NCORES = 8
P = 128                    # points per chunk / partition dim
SUBW = 20                  # sub-slot width (G columns)
SPB = 25                   # sub-slots per 512-col PSUM bank (25*20=500)
BANKW = 512
SEP_W, T_PEN, R_PEN = 1.0, 0.1, 1.0
EPS = 1e-8
BIG = 1e30
TAU = 1e-5                 # prune margin
DEEP = 8.5                 # deep-interior skip: per-point err <= e^-8.5
SENT = 1.0e6               # sentinel coordinate for padded points
UNROLL = 8                 # bodies per For_i iteration (timing loop only)

_NC_CACHE = {}


def _transform64(x, med, ang, tr):
    c, s = np.cos(ang), np.sin(ang)
    xc = x[..., 0] - med[:, None, 0]
    yc = x[..., 1] - med[:, None, 1]
    px = c[:, None] * xc - s[:, None] * yc + (med[:, 0] + tr[:, 0])[:, None]
    py = s[:, None] * xc + c[:, None] * yc + (med[:, 1] + tr[:, 1])[:, None]
    return np.stack([px, py], -1)


def _host_coeffs(ph, med, ang, tr, hm):
    """G[i] = A_i @ W: (C, 3, C, H) float64; rows act on raw [x, y, 1].

    W is orientation-normalized so that hull interiors have s > 0."""
    hulT = _transform64(ph, med, ang, tr)
    hx, hy = hulT[..., 0], hulT[..., 1]
    ex = np.roll(hx, -1, axis=1) - hx
    ey = np.roll(hy, -1, axis=1) - hy
    elen_raw = np.sqrt(ex * ex + ey * ey)
    elen = elen_raw + EPS
    evalid = elen_raw > 1e-6
    a = ex / elen
    b = -ey / elen
    d = -(ex * hy - ey * hx) / elen

    W = np.stack([b, a, d], axis=0)  # (3, C, H): coeffs on transformed [x,y,1]
    degenerate = np.zeros(C, bool)
    for j in range(C):
        inv = ~evalid[j]
        val = np.nonzero(evalid[j])[0]
        if inv.any():
            if len(val) > 0:
                W[:, j, inv] = W[:, j, val[-1]][:, None]
            else:
                W[:, j, :] = np.array([0.0, 0.0, BIG])[:, None]
                degenerate[j] = True
        if not degenerate[j]:
            vm = hm[j] if hm[j].any() else np.ones(H, bool)
            cx, cy = hulT[j, vm, 0].mean(), hulT[j, vm, 1].mean()
            sc = W[0, j, val] * cx + W[1, j, val] * cy + W[2, j, val]
            if np.median(sc) < 0:
                W[:, j, :] = -W[:, j, :]

    c, s = np.cos(ang), np.sin(ang)
    A = np.zeros((C, 3, 3))
    A[:, 0, 0] = c
    A[:, 0, 1] = s
    A[:, 1, 0] = -s
    A[:, 1, 1] = c
    A[:, 2, 0] = med[:, 0] + tr[:, 0] - c * med[:, 0] + s * med[:, 1]
    A[:, 2, 1] = med[:, 1] + tr[:, 1] - s * med[:, 0] - c * med[:, 1]
    A[:, 2, 2] = 1.0

    G = np.einsum("ikl,lm->ikm", A, W.reshape(3, C * H))
    return G.reshape(C, 3, C, H), hulT, degenerate


def _kd_split(p, ids, parts):
    """Split index array ids into `parts` groups of near-equal size (each
    <= ceil(len/parts)) by recursive median cuts on the wider dimension."""
    if parts == 1:
        return [ids]
    q = p[ids]
    dim = 0 if np.ptp(q[:, 0]) >= np.ptp(q[:, 1]) else 1
    order = ids[np.argsort(q[:, dim], kind="stable")]
    pl = parts // 2
    k = (len(order) * pl + parts - 1) // parts
    return _kd_split(p, order[:k], pl) + _kd_split(p, order[k:], parts - pl)


def _plan_and_pack(pc, ph, med, ang, tr, cm, hm):
    """Returns (cfg, in_maps); cfg = (k2, n1, host_deep)."""
    med64 = med.astype(np.float64)
    ang64 = ang.astype(np.float64)
    tr64 = tr.astype(np.float64)
    G, hulT, degen = _host_coeffs(ph.astype(np.float64), med64, ang64, tr64, hm)
    hull_ok = hm.sum(-1) >= 3
    hcnt = hm.sum(-1)

    host_deep = 0.0
    two_sub = []   # (i, chunk_idx_array, j) pairs with h > SUBW
    one_sub = []
    for i in range(C):
        valid = np.nonzero(cm[i])[0]
        if len(valid) == 0:
            continue
        parts = (len(valid) + P - 1) // P
        Gi = G[i].reshape(3, C * H)
        for ch in _kd_split(pc[i].astype(np.float64), valid, parts):
            q = pc[i, ch].astype(np.float64)
            qmin, qmax = q.min(0), q.max(0)
            corners = np.array(
                [[qmin[0], qmin[1], 1.0], [qmin[0], qmax[1], 1.0],
                 [qmax[0], qmin[1], 1.0], [qmax[0], qmax[1], 1.0]])
            sc = (corners @ Gi).reshape(4, C, H)
            neg_edge = (sc.max(0) < -TAU).any(-1)
            pos_edge = (sc.min(0) > TAU).any(-1)
            prunable = neg_edge & pos_edge
            env_lo_min = sc.min(-1).min(0)          # (C,) box-min of min_h s
            for j in range(C):
                if j == i or not hull_ok[j]:
                    continue
                if not degen[j] and prunable[j]:
                    continue
                if degen[j] or env_lo_min[j] >= DEEP:
                    host_deep += float(len(ch))
                    continue
                if hcnt[j] > SUBW:
                    two_sub.append((i, ch, j))
                else:
                    one_sub.append((i, ch, j))

    per2 = [two_sub[c::NCORES] for c in range(NCORES)]
    per1 = [one_sub[c::NCORES] for c in range(NCORES)]
    k2 = max(len(x) for x in per2)        # 2-sub pairs per core (padded)
    n1 = max(len(x) for x in per1)        # 1-sub pairs per core (padded)
    nsub = 2 * k2 + n1
    nbank = (nsub + SPB - 1) // SPB
    assert nbank <= 6, f"PSUM budget exceeded: nbank={nbank}"
    npair = k2 + n1

    in_maps = []
    for c in range(NCORES):
        # sub-slot s -> (pair, which half): [subA of 2-sub pairs | subB | 1-sub]
        subs = ([(t, 0) for t in per2[c]] + [(None, 0)] * (k2 - len(per2[c]))
                + [(t, 1) for t in per2[c]] + [(None, 0)] * (k2 - len(per2[c]))
                + [(t, 0) for t in per1[c]] + [(None, 0)] * (n1 - len(per1[c])))
        lhs = np.zeros((P, nbank * P), np.float32)
        rhs = np.zeros((P, nbank * BANKW), np.float32)
        cm3 = np.zeros((P, npair), np.float32)
        for b in range(nbank):
            tri = {}
            for si, (pair, half) in enumerate(subs[b * SPB:(b + 1) * SPB]):
                if pair is None:
                    continue
                i, ch, j = pair
                key = (i, ch.tobytes())
                if key not in tri:
                    t = tri[key] = len(tri)
                    n = len(ch)
                    lhs[3 * t + 0, b * P: b * P + n] = pc[i, ch, 0]
                    lhs[3 * t + 1, b * P: b * P + n] = pc[i, ch, 1]
                    lhs[3 * t + 0, b * P + n:(b + 1) * P] = SENT
                    lhs[3 * t + 1, b * P + n:(b + 1) * P] = SENT
                    lhs[3 * t + 2, b * P:(b + 1) * P] = 1.0
                t = tri[key]
                co = b * BANKW + si * SUBW
                rhs[3 * t: 3 * t + 3, co: co + SUBW] = \
                    G[i, :, j, half * SUBW:(half + 1) * SUBW]
        for pi, (i, ch, j) in enumerate(per2[c]):
            cm3[: len(ch), pi] = 1.0
        for pi, (i, ch, j) in enumerate(per1[c]):
            cm3[: len(ch), k2 + pi] = 1.0
        in_maps.append({
            "lhs": np.ascontiguousarray(lhs),
            "rhs": np.ascontiguousarray(rhs),
            "cmask": np.ascontiguousarray(cm3),
        })
    return (k2, n1, host_deep), in_maps


def _build_nc(cfg, reps=1, loop=None):
    import concourse.bacc as bacc
    import concourse.mybir as mybir
    from concourse.tile import TileContext

    k2, n1 = cfg[0], cfg[1]
    nsub = 2 * k2 + n1
    npair = k2 + n1
    nbank = (nsub + SPB - 1) // SPB
    f32 = mybir.dt.float32
    f32r = mybir.dt.float32r
    nc = bacc.Bacc()

    lhs_d = nc.dram_tensor("lhs", [P, nbank * P], f32r, kind="ExternalInput")
    rhs_d = nc.dram_tensor("rhs", [P, nbank * BANKW], f32r, kind="ExternalInput")
    cm_d = nc.dram_tensor("cmask", [P, npair], f32, kind="ExternalInput")
    out_d = nc.dram_tensor("out", [1, 1], f32, kind="ExternalOutput")

    # two PSUM groups (bufs=2 each) so matmuls overlap the other group's reduce
    gb1 = (nbank + 1) // 2
    groups = [list(range(0, gb1)), list(range(gb1, nbank))]
    groups = [g for g in groups if g]

    import os as _os
    unroll = int(_os.environ.get("UNROLL", str(UNROLL))) if loop is not None else 1

    with TileContext(nc) as tc:
        with tc.tile_pool(name="const", bufs=1) as cpool, \
             tc.tile_pool(name="work", bufs=2) as wpool, \
             tc.tile_pool(name="psum", bufs=2, space="PSUM") as ppool:

            sp = mybir.EngineType.SP
            lhs_sb = cpool.tile_from(lhs_d[:, :], forced_dma_engine=sp)
            rhs_sb = cpool.tile_from(rhs_d[:, :], forced_dma_engine=sp)
            cm_sb = cpool.tile_from(cm_d[:, :], forced_dma_engine=sp)
            vstrip = cpool.tile([P, npair], f32)
            ones_sb = cpool.tile([P, 1], f32)
            nc.vector.memset(ones_sb, 1.0)

            def body():
                mnsub = wpool.tile([P, nsub], f32, tag="mn")
                pairs_t = wpool.tile([P, npair], f32, tag="pair")
                w_t = wpool.tile([P, npair], f32, tag="w")
                gm_t = wpool.tile([P, npair], f32, tag="gm")
                for gi, banks in enumerate(groups):
                    gnb = len(banks)
                    ps = ppool.tile([P, gnb * BANKW], f32, tag=f"ps{gi}")
                    for li, b in enumerate(banks):
                        w = min(SPB, nsub - b * SPB) * SUBW
                        nc.tensor.matmul(
                            ps[:, li * BANKW: li * BANKW + w],
                            lhs_sb[0:3 * SPB, b * P:(b + 1) * P],
                            rhs_sb[0:3 * SPB, b * BANKW: b * BANKW + w],
                            start=True, stop=True,
                        )
                    # min-reduce: full banks as one 4D view + partial remainder
                    full = [b for b in banks if (b + 1) * SPB <= nsub]
                    so = banks[0] * SPB
                    if full:
                        view = ps.rearrange("p (b k) -> p b k", b=gnb) \
                            [:, 0:len(full), 0:SPB * SUBW] \
                            .rearrange("p b (s h) -> p b s h", h=SUBW)
                        nc.vector.tensor_reduce(
                            out=mnsub[:, so:so + len(full) * SPB], in_=view,
                            axis=mybir.AxisListType.X, op=mybir.AluOpType.min,
                        )
                    if len(full) < gnb:
                        rem = nsub - banks[len(full)] * SPB
                        rview = ps[:, len(full) * BANKW:
                                   len(full) * BANKW + rem * SUBW] \
                            .rearrange("p (s h) -> p s h", h=SUBW)
                        nc.vector.tensor_reduce(
                            out=mnsub[:, banks[len(full)] * SPB:
                                      banks[len(full)] * SPB + rem],
                            in_=rview,
                            axis=mybir.AxisListType.X, op=mybir.AluOpType.min,
                        )
                # pair strip: DVE combines 2-sub pairs, ACT copies 1-sub part
                # (Pool ISA only supports add/mult-type tensor ops)
                if k2 > 0:
                    nc.vector.tensor_tensor(
                        out=pairs_t[:, 0:k2], in0=mnsub[:, 0:k2],
                        in1=mnsub[:, k2:2 * k2], op=mybir.AluOpType.min)
                if n1 > 0:
                    nc.scalar.copy(
                        out=pairs_t[:, k2:npair], in_=mnsub[:, 2 * k2:nsub])
                nc.scalar.activation(
                    out=w_t, in_=pairs_t,
                    func=mybir.ActivationFunctionType.Sigmoid)
                nc.vector.tensor_scalar(
                    out=gm_t, in0=pairs_t, scalar1=-float(EPS), scalar2=None,
                    op0=mybir.AluOpType.is_ge)
                nc.gpsimd.tensor_tensor(
                    out=gm_t, in0=gm_t, in1=cm_sb, op=mybir.AluOpType.mult)
                nc.gpsimd.tensor_tensor(
                    out=vstrip, in0=w_t, in1=gm_t, op=mybir.AluOpType.mult)

            if loop is not None:
                stg = _os.environ.get("LOOP_STAGGERED", "0") == "1"
                with tc.For_i(0, loop, 1, staggered_reset=stg) as _i:
                    for _ in range(unroll):
                        body()
            else:
                for _ in range(reps):
                    body()

            acc = cpool.tile([P, 1], f32)
            nc.vector.tensor_reduce(
                out=acc, in_=vstrip, axis=mybir.AxisListType.X,
                op=mybir.AluOpType.add,
            )
            out_ps = ppool.tile([1, 1], f32, tag="ps2")
            nc.tensor.matmul(out_ps, acc, ones_sb, start=True, stop=True)
            out_sb = cpool.tile([1, 1], f32)
            nc.scalar.copy(out=out_sb, in_=out_ps)
            nc.sync.dma_start(out=out_d[:, :], in_=out_sb)

    nc.compile()
    return nc


def kernel(padded_clusters, padded_hulls, medoids, rotation_angles,
           translations, cluster_masks, hull_masks):
    pc = np.asarray(padded_clusters, dtype=np.float32)
    ph = np.asarray(padded_hulls, dtype=np.float32)
    med = np.asarray(medoids, dtype=np.float32)
    ang = np.asarray(rotation_angles, dtype=np.float32)
    tr = np.asarray(translations, dtype=np.float32)
    cm = np.asarray(cluster_masks)
    hm = np.asarray(hull_masks)

    cfg, in_maps = _plan_and_pack(pc, ph, med, ang, tr, cm, hm)

    key = ("nc", cfg[0], cfg[1])
    if key not in _NC_CACHE:
        _NC_CACHE[key] = _build_nc(cfg)
    nc = _NC_CACHE[key]

    from concourse.bass_utils import run_bass_kernel_spmd
    res = run_bass_kernel_spmd(nc, in_maps, core_ids=list(range(NCORES)))
    _NC_CACHE["last_results"] = res

    sep = sum(float(r["out"][0, 0]) for r in res.results) + cfg[2]
    total = (SEP_W * sep
             + T_PEN * float(np.sum(tr.astype(np.float64) ** 2))
             + R_PEN * float(np.sum(ang.astype(np.float64) ** 2)))
    return np.asarray(total, dtype=np.float32)
